# revision 21
# baseline (speedup 1.0000x reference)
"""AGC loss kernel for 8 Trainium2 NeuronCores (Bass/Tile).

Self-contained: builds the Bass program, shards inputs host-side, runs via
run_bass_kernel_spmd, returns the full (scalar) output.

Device mapping: target rows p sharded 800/core; sim = bf16 matmul with f32
PSUM; occurrence term folded into the matmul via two extra bf16 contraction
rows (hi/lo split); exact first-index argmax via max/max_index on f32 sim.
"""
import os
import sys
import numpy as np

for _p in ("/opt/trn_rl_repo", os.environ.get("TRN_RL_REPO", "")):
    if _p and _p not in sys.path and os.path.isdir(_p):
        sys.path.insert(0, _p)

import concourse.bass as bass
import concourse.mybir as mybir
from concourse import bacc, tile
from concourse import tile_utils

F32 = mybir.dt.float32
BF16 = mybir.dt.bfloat16
I16 = mybir.dt.int16
I32 = mybir.dt.int32
U32 = mybir.dt.uint32
ALU = mybir.AluOpType
ACTF = mybir.ActivationFunctionType
AXX = mybir.AxisListType.X

HO = 169                 # patch grid 169x169
NPOS = HO * HO           # 28561
C = 49                   # channels (7x7 patch)
KA = 51                  # + 2 aug rows (occ hi/lo)
Q = 6400                 # refer samples (columns)
NCORES = 8
ROWS = Q // NCORES       # 800 target rows per core
CHUNKS = [(i * 128, min(128, ROWS - i * 128)) for i in range((ROWS + 127) // 128)]
NCH = len(CHUNKS)
LAM2 = 0.1               # 2 * LAMBDA_OCC
RNE = float(2 ** 23)

# channel order grouped by e = dx % 3 (must match nothing host-side; any
# consistent permutation works since all downstream ops reduce over channels)
E_GROUPS = [(e, [dx for dx in range(7) if dx % 3 == e]) for e in range(3)]
CNT_E = [171, 171, 170]  # compacted column count per e


def _ts(eng, out, in0, s1, op0, s2=None, op1=None):
    return eng.tensor_scalar(out, in0, s1, s2,
                             op0, op1 if op1 is not None else ALU.bypass)


def build_program(debug_taps=False):
    tile_utils.max_sbuf_usage = 204 * 1024
    nc = bacc.Bacc("TRN2", target_bir_lowering=False, debug=False,
                   num_devices=NCORES)
    dbg = {}
    if debug_taps:
        dbg = {
            "d_gidx": nc.dram_tensor("d_gidx", [64, 400], F32, kind="ExternalOutput"),
            "d_feat": nc.dram_tensor("d_feat", [64, 7200], F32, kind="ExternalOutput"),
            "d_F": nc.dram_tensor("d_F", [64, 7200], F32, kind="ExternalOutput"),
            "d_s0": nc.dram_tensor("d_s0", [128, Q], F32, kind="ExternalOutput"),
            "d_idx": nc.dram_tensor("d_idx", [128, 16], F32, kind="ExternalOutput"),
            "d_cnt": nc.dram_tensor("d_cnt", [128, 50], F32, kind="ExternalOutput"),
            "d_sc": nc.dram_tensor("d_sc", [128, 96], F32, kind="ExternalOutput"),
            "d_ym": nc.dram_tensor("d_ym", [64, 1], F32, kind="ExternalOutput"),
            "d_n2": nc.dram_tensor("d_n2", [1, 7296], F32, kind="ExternalOutput"),
            "d_rs": nc.dram_tensor("d_rs", [128, 57], F32, kind="ExternalOutput"),
        }

    timg = nc.dram_tensor("timg", [512, 512], F32, kind="ExternalInput")
    rimg = nc.dram_tensor("rimg", [512, 512], F32, kind="ExternalInput")
    tfield = nc.dram_tensor("tfield", [ROWS, 2], F32, kind="ExternalInput")
    rfield = nc.dram_tensor("rfield", [Q, 2], F32, kind="ExternalInput")
    out_t = nc.dram_tensor("out", [1, 1], F32, kind="ExternalOutput")

    ce_hbm = [nc.dram_tensor(f"ce{i}", [512 * CNT_E[i]], F32) for i in range(3)]
    cc_in = nc.dram_tensor("cc_in", [Q], F32)
    cc_out = nc.dram_tensor("cc_out", [Q], F32, addr_space="Shared")
    ls_in = nc.dram_tensor("ls_in", [1], F32)
    ls_out = nc.dram_tensor("ls_out", [1], F32, addr_space="Shared")
    groups = [list(range(NCORES))]

    with tile.TileContext(nc) as tc:
        with tc.tile_pool(name="persist", bufs=1) as pp:
            F = pp.tile([64, 800 + Q], BF16, tag="F")        # xf | yf (+aug rows)
            iotaA = pp.tile([128, 50], F32, tag="iA")
            iotaB = pp.tile([128, 128], F32, tag="iB")
            ones_r = pp.tile([1, 64], F32, tag="ones_r")     # K=1 bcast lhsT (f32)
            ones_n2 = pp.tile([64, 1], BF16, tag="ones_n2")  # n2 lhsT (bf16)
            ones_c = pp.tile([128, 1], F32, tag="ones_c")
            loss_acc = pp.tile([128, 1], F32, tag="lacc")

            i32tmp = pp.tile([128, 128], I32, tag="i32tmp")
            nc.gpsimd.iota(i32tmp[:, 0:50], pattern=[[1, 50]], channel_multiplier=0)
            nc.vector.tensor_copy(iotaA[:], i32tmp[:, 0:50])
            nc.gpsimd.iota(i32tmp[:], pattern=[[1, 128]], channel_multiplier=0)
            nc.vector.tensor_copy(iotaB[:], i32tmp[:])
            nc.vector.memset(ones_r[:], 1.0)
            nc.vector.memset(ones_n2[:], 1.0)
            nc.vector.memset(ones_c[:], 1.0)
            nc.vector.memset(loss_acc[:], 0.0)
            # aug rows: zero rows 32-63 (rows 32-48 later overwritten by xf/yf
            # write), then ones into xf-aug via DMA (DVE can't start at p=49)
            nc.vector.memset(F[32:64, :], 0.0)
            aug2 = pp.tile([2, 800], BF16, tag="aug2")
            nc.vector.memset(aug2[:], 1.0)
            nc.sync.dma_start(F[49:51, 0:800], aug2[:])

            # =========== prologue: gather + normalize ====================
            with tc.tile_pool(name="pro", bufs=1) as pro:
                featcat = pro.tile([64, 800 + Q], F32, tag="featcat")

                with tc.tile_pool(name="progA", bufs=1) as gp:
                    apt = gp.tile([64, NPOS], F32, tag="apt")
                    nc.gpsimd.memset(apt[32:64, :], 0.0)

                    # ---- grid-sample indices for both fields ----
                    gidx = {}
                    for name, fld, n in (("t", tfield, ROWS), ("r", rfield, Q)):
                        nw = n // 16
                        wx = gp.tile([16, Q // 16], F32, tag="wx")
                        wy = gp.tile([16, Q // 16], F32, tag="wy")
                        fv = fld[:].rearrange("(j p) c -> p j c", p=16)
                        nc.sync.dma_start(wx[:, 0:nw], fv[:, :, 0:1].squeeze(2))
                        nc.sync.dma_start(wy[:, 0:nw], fv[:, :, 1:2].squeeze(2))
                        for w in (wx, wy):
                            u = w[:, 0:nw]
                            _ts(nc.vector, u, u, 2.0, ALU.mult, 1.0, ALU.subtract)
                            _ts(nc.vector, u, u, 1.0, ALU.add, float(HO), ALU.mult)
                            _ts(nc.vector, u, u, 1.0, ALU.subtract, 0.5, ALU.mult)
                            _ts(nc.vector, u, u, RNE, ALU.add)
                            _ts(nc.vector, u, u, RNE, ALU.subtract)
                            _ts(nc.vector, u, u, 0.0, ALU.max, float(HO - 1), ALU.min)
                        nc.vector.scalar_tensor_tensor(
                            wy[:, 0:nw], wy[:, 0:nw], float(HO), wx[:, 0:nw],
                            ALU.mult, ALU.add)
                        if dbg and name == "r":
                            nc.sync.dma_start(dbg["d_gidx"][0:16, :], wy[:, 0:nw])
                        gi = gp.tile([64, Q // 16], I16, tag=f"gi_{name}")
                        nc.vector.tensor_copy(gi[0:16, 0:nw], wy[:, 0:nw])
                        for k in range(1, 4):
                            nc.sync.dma_start(gi[16 * k:16 * k + 16, 0:nw],
                                              gi[0:16, 0:nw])
                        gidx[name] = (gi, nw)

                    # ---- per image: stage -> compact -> table -> gather ----
                    for img, iname, ncol, coff in ((timg, "t", ROWS, 0),
                                                   (rimg, "r", Q, 800)):
                        gi, nw = gidx[iname]
                        t_st = gp.tile([128, 4, 512], F32, tag="tst")
                        nc.sync.dma_start(
                            t_st[:], img[:].rearrange("(a b) w -> a b w", b=4))
                        for e in range(3):
                            cnt = CNT_E[e]
                            ce = gp.tile([128, 4, 171], F32, tag=f"ce{e}")
                            nc.vector.tensor_copy(
                                ce[:, :, 0:cnt],
                                t_st[:, :, e:e + 3 * (cnt - 1) + 1:3])
                            nc.sync.dma_start(
                                ce_hbm[e][:].rearrange("(a b c) -> a b c",
                                                       a=128, b=4),
                                ce[:, :, 0:cnt])
                        pbase = 0
                        for e, dxs in E_GROUPS:
                            cnt = CNT_E[e]
                            for dxq in range(len(dxs)):
                                src = bass.AP(
                                    tensor=ce_hbm[e], offset=dxq,
                                    ap=[[cnt, 7], [3 * cnt, HO], [1, HO]])
                                dst = apt[pbase:pbase + 7, :] \
                                    .rearrange("p (iy ix) -> p iy ix", iy=HO)
                                nc.gpsimd.dma_start(dst, src)
                                pbase += 7
                        nc.gpsimd.ap_gather(
                            out_ap=featcat[:, coff:coff + ncol].unsqueeze(2),
                            in_ap=apt[:].unsqueeze(2),
                            idxs_ap=gi[:, 0:nw],
                            channels=64, num_elems=NPOS, d=1, num_idxs=ncol)

                # ---- y_mean (refer), centered features ----
                if dbg:
                    nc.sync.dma_start(dbg["d_feat"][:], featcat[:])
                _gpcm = tc.tile_pool(name="progB", bufs=1)
                gp = _gpcm.__enter__()
                ymean = gp.tile([64, 1], F32, tag="ymean")
                nc.vector.memset(ymean[:], 0.0)
                nc.scalar.activation(featcat[0:C, 800:800 + Q],
                                     featcat[0:C, 800:800 + Q],
                                     ACTF.Copy, accum_out=ymean[0:C, :])
                if dbg:
                    nc.sync.dma_start(dbg["d_ym"][:], ymean[:])
                _ts(nc.vector, ymean[0:C, :], ymean[0:C, :], 1.0 / Q, ALU.mult)
                _ts(nc.vector, featcat[0:C, :], featcat[0:C, :],
                    ymean[0:C, :], ALU.subtract)

                # ---- column norms ----
                NT = 800 + Q
                sq = gp.tile([64, NT], BF16, tag="sq")
                nc.scalar.activation(sq[0:C, :], featcat[0:C, :], ACTF.Square)
                n2row = gp.tile([1, 7296], F32, tag="n2row")
                nc.vector.memset(n2row[:], 1.0)
                nsl = [(i * 512, min(512, NT - i * 512))
                       for i in range((NT + 511) // 512)]
                with tc.tile_pool(name="n2ps", bufs=2, space="PSUM") as n2p:
                    for r0 in range(0, len(nsl), 4):
                        sls = nsl[r0:r0 + 4]
                        ps = n2p.tile([1, 2048], F32, tag="n2psum")
                        for j, (o, w) in enumerate(sls):
                            nc.tensor.matmul(ps[:, j * 512:j * 512 + w],
                                             ones_n2[0:C, :], sq[0:C, o:o + w],
                                             start=True, stop=True)
                        o0 = sls[0][0]
                        wtot = sum(w for _, w in sls)
                        nc.scalar.activation(n2row[:, o0:o0 + wtot],
                                             ps[:, 0:wtot], ACTF.Copy)
                if dbg:
                    nc.sync.dma_start(dbg["d_n2"][:], n2row[:])
                # compact rsqrt with one Newton step
                cpt = gp.tile([128, 57], F32, tag="cpt")
                nc.sync.dma_start(
                    cpt[:], n2row[:].rearrange("a (p j) -> a p j", p=128))
                rc = gp.tile([128, 57], F32, tag="rc")
                nc.vector.reciprocal(rc[:], cpt[:])
                rs = gp.tile([128, 57], F32, tag="rs")
                nc.scalar.activation(rs[:], rc[:], ACTF.Sqrt)
                t2 = gp.tile([128, 57], F32, tag="t2")
                nc.vector.tensor_mul(t2[:], rs[:], rs[:])
                nc.vector.tensor_mul(t2[:], t2[:], cpt[:])
                _ts(nc.vector, t2[:], t2[:], -0.5, ALU.mult, 1.5, ALU.add)
                nc.vector.tensor_mul(rs[:], rs[:], t2[:])
                if dbg:
                    nc.sync.dma_start(dbg["d_rs"][:], rs[:])
                nc.sync.dma_start(
                    n2row[:].rearrange("a (p j) -> a p j", p=128), rs[:])
                # broadcast 1/norm to C partitions; write xf/yf bf16
                with tc.tile_pool(name="bcps", bufs=2, space="PSUM") as bp:
                    for o in range(0, NT, 2048):
                        w = min(2048, NT - o)
                        ps = bp.tile([64, 2048], F32, tag="bc")
                        for j in range(0, w, 512):
                            wj = min(512, w - j)
                            nc.tensor.matmul(ps[0:C, j:j + wj], ones_r[0:1, 0:C],
                                             n2row[:, o + j:o + j + wj],
                                             start=True, stop=True)
                        nc.vector.tensor_mul(F[0:C, o:o + w],
                                             featcat[0:C, o:o + w], ps[0:C, 0:w])
                if dbg:
                    nc.gpsimd.dma_start(dbg["d_F"][:], F[:])
                _gpcm.__exit__(None, None, None)

            # =========== main phases =====================================
            with tc.tile_pool(name="mid", bufs=1) as mp_:
                ohA = mp_.tile([128, NCH * 50], BF16, tag="ohA")
                ohB = mp_.tile([128, NCH * 128], BF16, tag="ohB")
                val8 = mp_.tile([128, 8], F32, tag="val8")
                idx8 = mp_.tile([128, 8], U32, tag="idx8")
                small = mp_.tile([128, 96], F32, tag="small")
                nc.vector.memset(small[:], 0.0)

                # ---- phase 1: sim matmul, f32 store, exact argmax ----
                with tc.tile_pool(name="ph1", bufs=2) as s1pool, \
                     tc.tile_pool(name="ph1ps", bufs=2, space="PSUM") as p1p:
                    for ci, (c0, pc) in enumerate(CHUNKS):
                        s_sl = s1pool.tile([128, Q], F32, tag="schunk")
                        for o in range(0, Q, 2048):
                            w = min(2048, Q - o)
                            ps = p1p.tile([128, 2048], F32, tag="s1")
                            for j in range(0, w, 512):
                                wj = min(512, w - j)
                                nc.tensor.matmul(
                                    ps[0:pc, j:j + wj], F[0:C, c0:c0 + pc],
                                    F[0:C, 800 + o + j:800 + o + j + wj],
                                    start=True, stop=True)
                            nc.scalar.activation(s_sl[0:pc, o:o + w],
                                                 ps[0:pc, 0:w], ACTF.Copy)
                        if dbg and ci == 0:
                            nc.sync.dma_start(dbg["d_s0"][:], s_sl[:])
                        nc.vector.max(val8[0:pc, :], s_sl[0:pc, :])
                        nc.vector.max_index(idx8[0:pc, :], val8[0:pc, :],
                                            s_sl[0:pc, :])
                        qf = small[:, ci:ci + 1]
                        nc.vector.tensor_copy(qf[0:pc, :], idx8[0:pc, 0:1])
                        af = small[:, 8 + ci:9 + ci]
                        _ts(nc.vector, af[0:pc, :], qf[0:pc, :], 1.0 / 128.0,
                            ALU.mult, 63.5 / 128.0, ALU.subtract)
                        _ts(nc.vector, af[0:pc, :], af[0:pc, :], RNE, ALU.add)
                        _ts(nc.vector, af[0:pc, :], af[0:pc, :], RNE, ALU.subtract)
                        bf_ = small[:, 16 + ci:17 + ci]
                        nc.vector.scalar_tensor_tensor(
                            bf_[0:pc, :], af[0:pc, :], -128.0, qf[0:pc, :],
                            ALU.mult, ALU.add)
                        _ts(nc.vector, ohA[0:pc, ci * 50:(ci + 1) * 50],
                            iotaA[0:pc, :], af[0:pc, :], ALU.is_equal)
                        _ts(nc.vector, ohB[0:pc, ci * 128:(ci + 1) * 128],
                            iotaB[0:pc, :], bf_[0:pc, :], ALU.is_equal)

                # ---- phase 2: histogram, all-reduce, occ rows ----
                with tc.tile_pool(name="ph2", bufs=1) as hp, \
                     tc.tile_pool(name="ph2ps", bufs=1, space="PSUM") as cp:
                    cpsum = cp.tile([64, 128], F32, tag="cpsum")
                    for ci, (c0, pc) in enumerate(CHUNKS):
                        nc.tensor.matmul(cpsum[0:50, :],
                                         ohA[0:pc, ci * 50:(ci + 1) * 50],
                                         ohB[0:pc, ci * 128:(ci + 1) * 128],
                                         start=(ci == 0), stop=(ci == NCH - 1))
                    csb = hp.tile([64, 128], F32, tag="csb")
                    nc.vector.tensor_copy(csb[0:50, :], cpsum[0:50, :])
                    nc.sync.dma_start(
                        cc_in[:].rearrange("(p j) -> p j", p=50), csb[0:50, :])
                    nc.gpsimd.collective_compute(
                        "AllReduce", ALU.add, replica_groups=groups,
                        ins=[cc_in[:].opt()], outs=[cc_out[:].opt()])
                    ccp = hp.tile([128, 50], F32, tag="ccp")
                    nc.sync.dma_start(ccp[:],
                                      cc_out[:].rearrange("(p j) -> p j", p=128))
                    if dbg:
                        nc.sync.dma_start(dbg["d_cnt"][:], ccp[:])
                    th = hp.tile([128, 50], F32, tag="th")
                    _ts(nc.vector, th[:], ccp[:], -LAM2, ALU.mult)
                    hh = hp.tile([128, 50], BF16, tag="hh")
                    nc.vector.tensor_copy(hh[:], th[:])
                    ll = hp.tile([128, 50], BF16, tag="ll")
                    nc.vector.tensor_sub(ll[:], th[:], hh[:])
                    nc.sync.dma_start(F[49:50, 800:800 + Q], hh[:])
                    nc.sync.dma_start(F[50:51, 800:800 + Q], ll[:])

                # ---- phase 3: s' matmuls, row min, exp-sum, loss ----
                SPW = 1024
                nsub = (Q + SPW - 1) // SPW
                wdump = mp_.tile([128, SPW], BF16, tag="wdump")
                mparts = mp_.tile([128, nsub], F32, tag="mparts")
                sparts = mp_.tile([128, nsub], F32, tag="sparts")
                with tc.tile_pool(name="ph3a", bufs=2, space="PSUM") as pa, \
                     tc.tile_pool(name="ph3b", bufs=2, space="PSUM") as pb:
                    for ci, (c0, pc) in enumerate(CHUNKS):
                        for si in range(nsub):
                            o = si * SPW
                            w = min(SPW, Q - o)
                            ps = pa.tile([128, SPW], F32, tag="s3")
                            for j in range(0, w, 512):
                                wj = min(512, w - j)
                                nc.tensor.matmul(
                                    ps[0:pc, j:j + wj], F[0:KA, c0:c0 + pc],
                                    F[0:KA, 800 + o + j:800 + o + j + wj],
                                    start=True, stop=True)
                            nc.vector.tensor_reduce(
                                mparts[0:pc, si:si + 1], ps[0:pc, 0:w],
                                axis=AXX, op=ALU.max)
                        mxp = small[:, 24 + ci:25 + ci]
                        nc.vector.tensor_reduce(mxp[0:pc, :], mparts[0:pc, :],
                                                axis=AXX, op=ALU.max)
                        m_ = small[:, 32 + ci:33 + ci]
                        _ts(nc.vector, m_[0:pc, :], mxp[0:pc, :], -0.5, ALU.mult,
                            0.5, ALU.add)
                        a_ = small[:, 40 + ci:41 + ci]
                        nc.vector.reciprocal(a_[0:pc, :], m_[0:pc, :])
                        b_ = small[:, 48 + ci:49 + ci]
                        _ts(nc.vector, b_[0:pc, :], a_[0:pc, :], -1.0, ALU.mult,
                            2.0, ALU.add)
                        logm = small[:, 56 + ci:57 + ci]
                        nc.vector.scalar_tensor_tensor(
                            logm[0:pc, :], mxp[0:pc, :], a_[0:pc, :], b_[0:pc, :],
                            ALU.mult, ALU.add)
                        for si in range(nsub):
                            o = si * SPW
                            w = min(SPW, Q - o)
                            ps = pb.tile([128, SPW], F32, tag="s4")
                            for j in range(0, w, 512):
                                wj = min(512, w - j)
                                nc.tensor.matmul(
                                    ps[0:pc, j:j + wj], F[0:KA, c0:c0 + pc],
                                    F[0:KA, 800 + o + j:800 + o + j + wj],
                                    start=True, stop=True)
                            nc.scalar.activation(
                                wdump[0:pc, 0:w], ps[0:pc, 0:w], ACTF.Exp,
                                bias=b_[0:pc, :], scale=a_[0:pc, :],
                                accum_out=sparts[0:pc, si:si + 1])
                        S_ = small[:, 64 + ci:65 + ci]
                        nc.vector.tensor_reduce(S_[0:pc, :], sparts[0:pc, :],
                                                axis=AXX, op=ALU.add)
                        lnS = small[:, 72 + ci:73 + ci]
                        nc.scalar.activation(lnS[0:pc, :], S_[0:pc, :], ACTF.Ln)
                        lc = small[:, 80 + ci:81 + ci]
                        nc.vector.tensor_sub(lc[0:pc, :], lnS[0:pc, :],
                                             logm[0:pc, :])
                        nc.vector.tensor_add(loss_acc[0:pc, :],
                                             loss_acc[0:pc, :], lc[0:pc, :])

                # ---- final reduce + all-reduce ----
                with tc.tile_pool(name="fin", bufs=1) as fp, \
                     tc.tile_pool(name="finps", bufs=1, space="PSUM") as fps:
                    tot = fps.tile([1, 1], F32, tag="tot")
                    nc.tensor.matmul(tot[:], loss_acc[:], ones_c[:],
                                     start=True, stop=True)
                    tsb = fp.tile([1, 1], F32, tag="tsb")
                    nc.vector.tensor_copy(tsb[:], tot[:])
                    nc.sync.dma_start(ls_in[:].unsqueeze(0), tsb[:])
                    nc.gpsimd.collective_compute(
                        "AllReduce", ALU.add, replica_groups=groups,
                        ins=[ls_in[:].opt()], outs=[ls_out[:].opt()])
                    res = fp.tile([1, 1], F32, tag="res")
                    nc.sync.dma_start(res[:], ls_out[:].unsqueeze(0))
                    if dbg:
                        nc.sync.dma_start(dbg["d_sc"][:], small[:])
                    _ts(nc.vector, res[:], res[:], 1.0 / Q, ALU.mult)
                    nc.sync.dma_start(out_t[:], res[:])

    nc.compile()
    return nc


_NC = None


def _get_nc():
    global _NC
    if _NC is None:
        _NC = build_program()
    return _NC


def make_in_maps(target_features, refer_features, target_field, refer_field):
    timg_np = np.ascontiguousarray(
        np.asarray(target_features, np.float32).reshape(512, 512))
    rimg_np = np.ascontiguousarray(
        np.asarray(refer_features, np.float32).reshape(512, 512))
    tf = np.ascontiguousarray(np.asarray(target_field, np.float32).reshape(-1, 2))
    rf = np.ascontiguousarray(np.asarray(refer_field, np.float32).reshape(-1, 2))
    in_maps = []
    for k in range(NCORES):
        in_maps.append({
            "timg": timg_np,
            "rimg": rimg_np,
            "tfield": np.ascontiguousarray(tf[k * ROWS:(k + 1) * ROWS]),
            "rfield": rf,
        })
    return in_maps


LAST_RESULTS = None


def kernel(target_features, refer_features, target_field, refer_field,
           args=None, **_ignored):
    global LAST_RESULTS
    from concourse import bass_utils
    nc = _get_nc()
    in_maps = make_in_maps(target_features, refer_features,
                           target_field, refer_field)
    res = bass_utils.run_bass_kernel_spmd(
        nc, in_maps, core_ids=list(range(NCORES)),
        trace=bool(int(os.environ.get("AGC_TRACE", "0"))))
    LAST_RESULTS = res
    return np.asarray(res.results[0]["out"], np.float32).reshape(())


if __name__ == "__main__":
    if "--build" in sys.argv:
        build_program()
        print("BUILD OK")


# revision 22
# speedup vs baseline: 1.0558x; 1.0558x over previous
"""AGC loss kernel for 8 Trainium2 NeuronCores (Bass/Tile).

Self-contained: builds the Bass program, shards inputs host-side, runs via
run_bass_kernel_spmd, returns the full (scalar) output.

Device mapping: target rows p sharded 800/core; sim = bf16 matmul with f32
PSUM; occurrence term folded into the matmul via two extra bf16 contraction
rows (hi/lo split); exact first-index argmax via max/max_index on f32 sim.
"""
import os
import sys
import numpy as np

for _p in ("/opt/trn_rl_repo", os.environ.get("TRN_RL_REPO", "")):
    if _p and _p not in sys.path and os.path.isdir(_p):
        sys.path.insert(0, _p)

import concourse.bass as bass
import concourse.mybir as mybir
from concourse import bacc, tile
from concourse import tile_utils

F32 = mybir.dt.float32
BF16 = mybir.dt.bfloat16
I16 = mybir.dt.int16
I32 = mybir.dt.int32
U32 = mybir.dt.uint32
ALU = mybir.AluOpType
ACTF = mybir.ActivationFunctionType
AXX = mybir.AxisListType.X

HO = 169                 # patch grid 169x169
NPOS = HO * HO           # 28561
C = 49                   # channels (7x7 patch)
KA = 51                  # + 2 aug rows (occ hi/lo)
Q = 6400                 # refer samples (columns)
NCORES = 8
ROWS = Q // NCORES       # 800 target rows per core
CHUNKS = [(i * 128, min(128, ROWS - i * 128)) for i in range((ROWS + 127) // 128)]
NCH = len(CHUNKS)
LAM2 = 0.1               # 2 * LAMBDA_OCC
RNE = float(2 ** 23)

# channel order grouped by e = dx % 3 (must match nothing host-side; any
# consistent permutation works since all downstream ops reduce over channels)
E_GROUPS = [(e, [dx for dx in range(7) if dx % 3 == e]) for e in range(3)]
CNT_E = [171, 171, 170]  # compacted column count per e


def _ts(eng, out, in0, s1, op0, s2=None, op1=None):
    return eng.tensor_scalar(out, in0, s1, s2,
                             op0, op1 if op1 is not None else ALU.bypass)


def build_program(debug_taps=False):
    tile_utils.max_sbuf_usage = 204 * 1024
    nc = bacc.Bacc("TRN2", target_bir_lowering=False, debug=False,
                   num_devices=NCORES)
    dbg = {}
    if debug_taps:
        dbg = {
            "d_gidx": nc.dram_tensor("d_gidx", [64, 400], F32, kind="ExternalOutput"),
            "d_feat": nc.dram_tensor("d_feat", [64, 7200], F32, kind="ExternalOutput"),
            "d_F": nc.dram_tensor("d_F", [64, 7200], F32, kind="ExternalOutput"),
            "d_s0": nc.dram_tensor("d_s0", [128, Q], F32, kind="ExternalOutput"),
            "d_idx": nc.dram_tensor("d_idx", [128, 16], F32, kind="ExternalOutput"),
            "d_cnt": nc.dram_tensor("d_cnt", [128, 50], F32, kind="ExternalOutput"),
            "d_sc": nc.dram_tensor("d_sc", [128, 96], F32, kind="ExternalOutput"),
            "d_ym": nc.dram_tensor("d_ym", [64, 1], F32, kind="ExternalOutput"),
            "d_n2": nc.dram_tensor("d_n2", [1, 7296], F32, kind="ExternalOutput"),
            "d_rs": nc.dram_tensor("d_rs", [128, 57], F32, kind="ExternalOutput"),
        }

    timg = nc.dram_tensor("timg", [512, 512], F32, kind="ExternalInput")
    rimg = nc.dram_tensor("rimg", [512, 512], F32, kind="ExternalInput")
    tfield = nc.dram_tensor("tfield", [ROWS, 2], F32, kind="ExternalInput")
    rfield = nc.dram_tensor("rfield", [Q, 2], F32, kind="ExternalInput")
    out_t = nc.dram_tensor("out", [1, 1], F32, kind="ExternalOutput")

    ce_hbm = [nc.dram_tensor(f"ce{i}", [512 * CNT_E[i]], F32) for i in range(3)]
    cc_in = nc.dram_tensor("cc_in", [Q], F32)
    cc_out = nc.dram_tensor("cc_out", [Q], F32, addr_space="Shared")
    ls_in = nc.dram_tensor("ls_in", [1], F32)
    ls_out = nc.dram_tensor("ls_out", [1], F32, addr_space="Shared")
    groups = [list(range(NCORES))]

    with tile.TileContext(nc) as tc:
        with tc.tile_pool(name="persist", bufs=1) as pp:
            F = pp.tile([64, 800 + Q], BF16, tag="F")        # xf | yf (+aug rows)
            iotaA = pp.tile([128, 50], F32, tag="iA")
            iotaB = pp.tile([128, 128], F32, tag="iB")
            ones_r = pp.tile([1, 64], F32, tag="ones_r")     # K=1 bcast lhsT (f32)
            ones_n2 = pp.tile([64, 1], BF16, tag="ones_n2")  # n2 lhsT (bf16)
            ones_c = pp.tile([128, 1], F32, tag="ones_c")
            loss_acc = pp.tile([128, 1], F32, tag="lacc")

            i32tmp = pp.tile([128, 128], I32, tag="i32tmp")
            nc.gpsimd.iota(i32tmp[:, 0:50], pattern=[[1, 50]], channel_multiplier=0)
            nc.vector.tensor_copy(iotaA[:], i32tmp[:, 0:50])
            nc.gpsimd.iota(i32tmp[:], pattern=[[1, 128]], channel_multiplier=0)
            nc.vector.tensor_copy(iotaB[:], i32tmp[:])
            nc.vector.memset(ones_r[:], 1.0)
            nc.vector.memset(ones_n2[:], 1.0)
            nc.vector.memset(ones_c[:], 1.0)
            nc.vector.memset(loss_acc[:], 0.0)
            # aug rows: zero rows 32-63 (rows 32-48 later overwritten by xf/yf
            # write), then ones into xf-aug via DMA (DVE can't start at p=49)
            nc.vector.memset(F[32:64, :], 0.0)
            aug2 = pp.tile([2, 800], BF16, tag="aug2")
            nc.vector.memset(aug2[:], 1.0)
            nc.sync.dma_start(F[49:51, 0:800], aug2[:])

            # =========== prologue: gather + normalize ====================
            with tc.tile_pool(name="pro", bufs=1) as pro:
                featcat = pro.tile([64, 800 + Q], F32, tag="featcat")

                with tc.tile_pool(name="progA", bufs=1) as gp:
                    apt = gp.tile([64, NPOS], F32, tag="apt")
                    nc.gpsimd.memset(apt[32:64, :], 0.0)

                    # ---- grid-sample indices for both fields ----
                    gidx = {}
                    for name, fld, n in (("t", tfield, ROWS), ("r", rfield, Q)):
                        nw = n // 16
                        wx = gp.tile([16, Q // 16], F32, tag="wx")
                        wy = gp.tile([16, Q // 16], F32, tag="wy")
                        fv = fld[:].rearrange("(j p) c -> p j c", p=16)
                        nc.sync.dma_start(wx[:, 0:nw], fv[:, :, 0:1].squeeze(2))
                        nc.sync.dma_start(wy[:, 0:nw], fv[:, :, 1:2].squeeze(2))
                        for w in (wx, wy):
                            u = w[:, 0:nw]
                            _ts(nc.vector, u, u, 2.0, ALU.mult, 1.0, ALU.subtract)
                            _ts(nc.vector, u, u, 1.0, ALU.add, float(HO), ALU.mult)
                            _ts(nc.vector, u, u, 1.0, ALU.subtract, 0.5, ALU.mult)
                            _ts(nc.vector, u, u, RNE, ALU.add)
                            _ts(nc.vector, u, u, RNE, ALU.subtract)
                            _ts(nc.vector, u, u, 0.0, ALU.max, float(HO - 1), ALU.min)
                        nc.vector.scalar_tensor_tensor(
                            wy[:, 0:nw], wy[:, 0:nw], float(HO), wx[:, 0:nw],
                            ALU.mult, ALU.add)
                        if dbg and name == "r":
                            nc.sync.dma_start(dbg["d_gidx"][0:16, :], wy[:, 0:nw])
                        gi = gp.tile([64, Q // 16], I16, tag=f"gi_{name}")
                        nc.vector.tensor_copy(gi[0:16, 0:nw], wy[:, 0:nw])
                        for k in range(1, 4):
                            nc.sync.dma_start(gi[16 * k:16 * k + 16, 0:nw],
                                              gi[0:16, 0:nw])
                        gidx[name] = (gi, nw)

                    # ---- per image: stage -> compact -> table -> gather ----
                    for img, iname, ncol, coff in ((timg, "t", ROWS, 0),
                                                   (rimg, "r", Q, 800)):
                        gi, nw = gidx[iname]
                        t_st = gp.tile([128, 4, 512], F32, tag="tst")
                        nc.sync.dma_start(
                            t_st[:], img[:].rearrange("(a b) w -> a b w", b=4))
                        for e in range(3):
                            cnt = CNT_E[e]
                            ce = gp.tile([128, 4, 171], F32, tag=f"ce{e}")
                            nc.vector.tensor_copy(
                                ce[:, :, 0:cnt],
                                t_st[:, :, e:e + 3 * (cnt - 1) + 1:3])
                            nc.sync.dma_start(
                                ce_hbm[e][:].rearrange("(a b c) -> a b c",
                                                       a=128, b=4),
                                ce[:, :, 0:cnt])
                        pbase = 0
                        qi = 0
                        for e, dxs in E_GROUPS:
                            cnt = CNT_E[e]
                            for dxq in range(len(dxs)):
                                src = bass.AP(
                                    tensor=ce_hbm[e], offset=dxq,
                                    ap=[[cnt, 7], [3 * cnt, HO], [1, HO]])
                                dst = apt[pbase:pbase + 7, :] \
                                    .rearrange("p (iy ix) -> p iy ix", iy=HO)
                                eng = nc.sync if qi % 2 == 0 else nc.scalar
                                eng.dma_start(dst, src)
                                qi += 1
                                pbase += 7
                        nc.gpsimd.ap_gather(
                            out_ap=featcat[:, coff:coff + ncol].unsqueeze(2),
                            in_ap=apt[:].unsqueeze(2),
                            idxs_ap=gi[:, 0:nw],
                            channels=64, num_elems=NPOS, d=1, num_idxs=ncol)

                # ---- y_mean (refer), centered features ----
                if dbg:
                    nc.sync.dma_start(dbg["d_feat"][:], featcat[:])
                _gpcm = tc.tile_pool(name="progB", bufs=1)
                gp = _gpcm.__enter__()
                ymean = gp.tile([64, 1], F32, tag="ymean")
                nc.vector.memset(ymean[:], 0.0)
                nc.scalar.activation(featcat[0:C, 800:800 + Q],
                                     featcat[0:C, 800:800 + Q],
                                     ACTF.Copy, accum_out=ymean[0:C, :])
                if dbg:
                    nc.sync.dma_start(dbg["d_ym"][:], ymean[:])
                _ts(nc.vector, ymean[0:C, :], ymean[0:C, :], 1.0 / Q, ALU.mult)
                _ts(nc.vector, featcat[0:C, :], featcat[0:C, :],
                    ymean[0:C, :], ALU.subtract)

                # ---- column norms ----
                NT = 800 + Q
                sq = gp.tile([64, NT], BF16, tag="sq")
                nc.scalar.activation(sq[0:C, :], featcat[0:C, :], ACTF.Square)
                n2row = gp.tile([1, 7296], F32, tag="n2row")
                nc.vector.memset(n2row[:], 1.0)
                nsl = [(i * 512, min(512, NT - i * 512))
                       for i in range((NT + 511) // 512)]
                with tc.tile_pool(name="n2ps", bufs=2, space="PSUM") as n2p:
                    for r0 in range(0, len(nsl), 4):
                        sls = nsl[r0:r0 + 4]
                        ps = n2p.tile([1, 2048], F32, tag="n2psum")
                        for j, (o, w) in enumerate(sls):
                            nc.tensor.matmul(ps[:, j * 512:j * 512 + w],
                                             ones_n2[0:C, :], sq[0:C, o:o + w],
                                             start=True, stop=True)
                        o0 = sls[0][0]
                        wtot = sum(w for _, w in sls)
                        nc.scalar.activation(n2row[:, o0:o0 + wtot],
                                             ps[:, 0:wtot], ACTF.Copy)
                if dbg:
                    nc.sync.dma_start(dbg["d_n2"][:], n2row[:])
                # compact rsqrt with one Newton step
                cpt = gp.tile([128, 57], F32, tag="cpt")
                nc.sync.dma_start(
                    cpt[:], n2row[:].rearrange("a (p j) -> a p j", p=128))
                rc = gp.tile([128, 57], F32, tag="rc")
                nc.vector.reciprocal(rc[:], cpt[:])
                rs = gp.tile([128, 57], F32, tag="rs")
                nc.scalar.activation(rs[:], rc[:], ACTF.Sqrt)
                t2 = gp.tile([128, 57], F32, tag="t2")
                nc.vector.tensor_mul(t2[:], rs[:], rs[:])
                nc.vector.tensor_mul(t2[:], t2[:], cpt[:])
                _ts(nc.vector, t2[:], t2[:], -0.5, ALU.mult, 1.5, ALU.add)
                nc.vector.tensor_mul(rs[:], rs[:], t2[:])
                if dbg:
                    nc.sync.dma_start(dbg["d_rs"][:], rs[:])
                nc.sync.dma_start(
                    n2row[:].rearrange("a (p j) -> a p j", p=128), rs[:])
                # broadcast 1/norm to C partitions; write xf/yf bf16
                with tc.tile_pool(name="bcps", bufs=2, space="PSUM") as bp:
                    for o in range(0, NT, 2048):
                        w = min(2048, NT - o)
                        ps = bp.tile([64, 2048], F32, tag="bc")
                        for j in range(0, w, 512):
                            wj = min(512, w - j)
                            nc.tensor.matmul(ps[0:C, j:j + wj], ones_r[0:1, 0:C],
                                             n2row[:, o + j:o + j + wj],
                                             start=True, stop=True)
                        nc.vector.tensor_mul(F[0:C, o:o + w],
                                             featcat[0:C, o:o + w], ps[0:C, 0:w])
                if dbg:
                    nc.gpsimd.dma_start(dbg["d_F"][:], F[:])
                _gpcm.__exit__(None, None, None)

            # =========== main phases =====================================
            with tc.tile_pool(name="mid", bufs=1) as mp_:
                ohA = mp_.tile([128, NCH * 50], BF16, tag="ohA")
                ohB = mp_.tile([128, NCH * 128], BF16, tag="ohB")
                val8 = mp_.tile([128, 8], F32, tag="val8")
                idx8 = mp_.tile([128, 8], U32, tag="idx8")
                small = mp_.tile([128, 96], F32, tag="small")
                nc.vector.memset(small[:], 0.0)

                # ---- phase 1: sim matmul, f32 store, exact argmax ----
                with tc.tile_pool(name="ph1", bufs=2) as s1pool, \
                     tc.tile_pool(name="ph1ps", bufs=2, space="PSUM") as p1p:
                    for ci, (c0, pc) in enumerate(CHUNKS):
                        s_sl = s1pool.tile([128, Q], F32, tag="schunk")
                        for o in range(0, Q, 2048):
                            w = min(2048, Q - o)
                            ps = p1p.tile([128, 2048], F32, tag="s1")
                            for j in range(0, w, 512):
                                wj = min(512, w - j)
                                nc.tensor.matmul(
                                    ps[0:pc, j:j + wj], F[0:C, c0:c0 + pc],
                                    F[0:C, 800 + o + j:800 + o + j + wj],
                                    start=True, stop=True)
                            nc.scalar.activation(s_sl[0:pc, o:o + w],
                                                 ps[0:pc, 0:w], ACTF.Copy)
                        if dbg and ci == 0:
                            nc.sync.dma_start(dbg["d_s0"][:], s_sl[:])
                        nc.vector.max(val8[0:pc, :], s_sl[0:pc, :])
                        nc.vector.max_index(idx8[0:pc, :], val8[0:pc, :],
                                            s_sl[0:pc, :])
                        qf = small[:, ci:ci + 1]
                        nc.vector.tensor_copy(qf[0:pc, :], idx8[0:pc, 0:1])
                        af = small[:, 8 + ci:9 + ci]
                        _ts(nc.vector, af[0:pc, :], qf[0:pc, :], 1.0 / 128.0,
                            ALU.mult, 63.5 / 128.0, ALU.subtract)
                        _ts(nc.vector, af[0:pc, :], af[0:pc, :], RNE, ALU.add)
                        _ts(nc.vector, af[0:pc, :], af[0:pc, :], RNE, ALU.subtract)
                        bf_ = small[:, 16 + ci:17 + ci]
                        nc.vector.scalar_tensor_tensor(
                            bf_[0:pc, :], af[0:pc, :], -128.0, qf[0:pc, :],
                            ALU.mult, ALU.add)
                        _ts(nc.vector, ohA[0:pc, ci * 50:(ci + 1) * 50],
                            iotaA[0:pc, :], af[0:pc, :], ALU.is_equal)
                        _ts(nc.vector, ohB[0:pc, ci * 128:(ci + 1) * 128],
                            iotaB[0:pc, :], bf_[0:pc, :], ALU.is_equal)

                # ---- phase 2: histogram, all-reduce, occ rows ----
                with tc.tile_pool(name="ph2", bufs=1) as hp, \
                     tc.tile_pool(name="ph2ps", bufs=1, space="PSUM") as cp:
                    cpsum = cp.tile([64, 128], F32, tag="cpsum")
                    for ci, (c0, pc) in enumerate(CHUNKS):
                        nc.tensor.matmul(cpsum[0:50, :],
                                         ohA[0:pc, ci * 50:(ci + 1) * 50],
                                         ohB[0:pc, ci * 128:(ci + 1) * 128],
                                         start=(ci == 0), stop=(ci == NCH - 1))
                    csb = hp.tile([64, 128], F32, tag="csb")
                    nc.vector.tensor_copy(csb[0:50, :], cpsum[0:50, :])
                    nc.sync.dma_start(
                        cc_in[:].rearrange("(p j) -> p j", p=50), csb[0:50, :])
                    nc.gpsimd.collective_compute(
                        "AllReduce", ALU.add, replica_groups=groups,
                        ins=[cc_in[:].opt()], outs=[cc_out[:].opt()])
                    ccp = hp.tile([128, 50], F32, tag="ccp")
                    nc.sync.dma_start(ccp[:],
                                      cc_out[:].rearrange("(p j) -> p j", p=128))
                    if dbg:
                        nc.sync.dma_start(dbg["d_cnt"][:], ccp[:])
                    th = hp.tile([128, 50], F32, tag="th")
                    _ts(nc.vector, th[:], ccp[:], -LAM2, ALU.mult)
                    hh = hp.tile([128, 50], BF16, tag="hh")
                    nc.vector.tensor_copy(hh[:], th[:])
                    ll = hp.tile([128, 50], BF16, tag="ll")
                    nc.vector.tensor_sub(ll[:], th[:], hh[:])
                    nc.sync.dma_start(F[49:50, 800:800 + Q], hh[:])
                    nc.sync.dma_start(F[50:51, 800:800 + Q], ll[:])

                # ---- phase 3: s' matmuls, row min, exp-sum, loss ----
                SPW = 1024
                nsub = (Q + SPW - 1) // SPW
                wdump = mp_.tile([128, SPW], BF16, tag="wdump")
                mparts = mp_.tile([128, nsub], F32, tag="mparts")
                sparts = mp_.tile([128, nsub], F32, tag="sparts")
                with tc.tile_pool(name="ph3a", bufs=2, space="PSUM") as pa, \
                     tc.tile_pool(name="ph3b", bufs=2, space="PSUM") as pb:
                    for ci, (c0, pc) in enumerate(CHUNKS):
                        for si in range(nsub):
                            o = si * SPW
                            w = min(SPW, Q - o)
                            ps = pa.tile([128, SPW], F32, tag="s3")
                            for j in range(0, w, 512):
                                wj = min(512, w - j)
                                nc.tensor.matmul(
                                    ps[0:pc, j:j + wj], F[0:KA, c0:c0 + pc],
                                    F[0:KA, 800 + o + j:800 + o + j + wj],
                                    start=True, stop=True)
                            nc.vector.tensor_reduce(
                                mparts[0:pc, si:si + 1], ps[0:pc, 0:w],
                                axis=AXX, op=ALU.max)
                        mxp = small[:, 24 + ci:25 + ci]
                        nc.vector.tensor_reduce(mxp[0:pc, :], mparts[0:pc, :],
                                                axis=AXX, op=ALU.max)
                        m_ = small[:, 32 + ci:33 + ci]
                        _ts(nc.vector, m_[0:pc, :], mxp[0:pc, :], -0.5, ALU.mult,
                            0.5, ALU.add)
                        a_ = small[:, 40 + ci:41 + ci]
                        nc.vector.reciprocal(a_[0:pc, :], m_[0:pc, :])
                        b_ = small[:, 48 + ci:49 + ci]
                        _ts(nc.vector, b_[0:pc, :], a_[0:pc, :], -1.0, ALU.mult,
                            2.0, ALU.add)
                        logm = small[:, 56 + ci:57 + ci]
                        nc.vector.scalar_tensor_tensor(
                            logm[0:pc, :], mxp[0:pc, :], a_[0:pc, :], b_[0:pc, :],
                            ALU.mult, ALU.add)
                        for si in range(nsub):
                            o = si * SPW
                            w = min(SPW, Q - o)
                            ps = pb.tile([128, SPW], F32, tag="s4")
                            for j in range(0, w, 512):
                                wj = min(512, w - j)
                                nc.tensor.matmul(
                                    ps[0:pc, j:j + wj], F[0:KA, c0:c0 + pc],
                                    F[0:KA, 800 + o + j:800 + o + j + wj],
                                    start=True, stop=True)
                            nc.scalar.activation(
                                wdump[0:pc, 0:w], ps[0:pc, 0:w], ACTF.Exp,
                                bias=b_[0:pc, :], scale=a_[0:pc, :],
                                accum_out=sparts[0:pc, si:si + 1])
                        S_ = small[:, 64 + ci:65 + ci]
                        nc.vector.tensor_reduce(S_[0:pc, :], sparts[0:pc, :],
                                                axis=AXX, op=ALU.add)
                        lnS = small[:, 72 + ci:73 + ci]
                        nc.scalar.activation(lnS[0:pc, :], S_[0:pc, :], ACTF.Ln)
                        lc = small[:, 80 + ci:81 + ci]
                        nc.vector.tensor_sub(lc[0:pc, :], lnS[0:pc, :],
                                             logm[0:pc, :])
                        nc.vector.tensor_add(loss_acc[0:pc, :],
                                             loss_acc[0:pc, :], lc[0:pc, :])

                # ---- final reduce + all-reduce ----
                with tc.tile_pool(name="fin", bufs=1) as fp, \
                     tc.tile_pool(name="finps", bufs=1, space="PSUM") as fps:
                    tot = fps.tile([1, 1], F32, tag="tot")
                    nc.tensor.matmul(tot[:], loss_acc[:], ones_c[:],
                                     start=True, stop=True)
                    tsb = fp.tile([1, 1], F32, tag="tsb")
                    nc.vector.tensor_copy(tsb[:], tot[:])
                    nc.sync.dma_start(ls_in[:].unsqueeze(0), tsb[:])
                    nc.gpsimd.collective_compute(
                        "AllReduce", ALU.add, replica_groups=groups,
                        ins=[ls_in[:].opt()], outs=[ls_out[:].opt()])
                    res = fp.tile([1, 1], F32, tag="res")
                    nc.sync.dma_start(res[:], ls_out[:].unsqueeze(0))
                    if dbg:
                        nc.sync.dma_start(dbg["d_sc"][:], small[:])
                    _ts(nc.vector, res[:], res[:], 1.0 / Q, ALU.mult)
                    nc.sync.dma_start(out_t[:], res[:])

    nc.compile()
    return nc


_NC = None


def _get_nc():
    global _NC
    if _NC is None:
        _NC = build_program()
    return _NC


def make_in_maps(target_features, refer_features, target_field, refer_field):
    timg_np = np.ascontiguousarray(
        np.asarray(target_features, np.float32).reshape(512, 512))
    rimg_np = np.ascontiguousarray(
        np.asarray(refer_features, np.float32).reshape(512, 512))
    tf = np.ascontiguousarray(np.asarray(target_field, np.float32).reshape(-1, 2))
    rf = np.ascontiguousarray(np.asarray(refer_field, np.float32).reshape(-1, 2))
    in_maps = []
    for k in range(NCORES):
        in_maps.append({
            "timg": timg_np,
            "rimg": rimg_np,
            "tfield": np.ascontiguousarray(tf[k * ROWS:(k + 1) * ROWS]),
            "rfield": rf,
        })
    return in_maps


LAST_RESULTS = None


def kernel(target_features, refer_features, target_field, refer_field,
           args=None, **_ignored):
    global LAST_RESULTS
    from concourse import bass_utils
    nc = _get_nc()
    in_maps = make_in_maps(target_features, refer_features,
                           target_field, refer_field)
    res = bass_utils.run_bass_kernel_spmd(
        nc, in_maps, core_ids=list(range(NCORES)),
        trace=bool(int(os.environ.get("AGC_TRACE", "0"))))
    LAST_RESULTS = res
    return np.asarray(res.results[0]["out"], np.float32).reshape(())


if __name__ == "__main__":
    if "--build" in sys.argv:
        build_program()
        print("BUILD OK")


# revision 28
# speedup vs baseline: 1.1641x; 1.1026x over previous
"""AGC loss kernel for 8 Trainium2 NeuronCores (Bass/Tile).

Self-contained: builds the Bass program, shards inputs host-side, runs via
run_bass_kernel_spmd, returns the full (scalar) output.

Device mapping: target rows p sharded 800/core; sim = bf16 matmul with f32
PSUM; occurrence term folded into the matmul via two extra bf16 contraction
rows (hi/lo split); exact first-index argmax via max/max_index on f32 sim.
"""
import os
import sys
import numpy as np

for _p in ("/opt/trn_rl_repo", os.environ.get("TRN_RL_REPO", "")):
    if _p and _p not in sys.path and os.path.isdir(_p):
        sys.path.insert(0, _p)

import concourse.bass as bass
import concourse.mybir as mybir
from concourse import bacc, tile
from concourse import tile_utils

F32 = mybir.dt.float32
BF16 = mybir.dt.bfloat16
I16 = mybir.dt.int16
I32 = mybir.dt.int32
U32 = mybir.dt.uint32
ALU = mybir.AluOpType
ACTF = mybir.ActivationFunctionType
AXX = mybir.AxisListType.X

HO = 169                 # patch grid 169x169
GRP = 43 * 684           # residue-group stride in the grouped image (29412)
NPOS = 171 * 168 + 168 + 1   # 28897 table elems per channel (g = 171*iy + ix)
C = 49                   # channels (7x7 patch)
KA = 51                  # + 2 aug rows (occ hi/lo)
Q = 6400                 # refer samples (columns)
NCORES = 8
ROWS = Q // NCORES       # 800 target rows per core
CHUNKS = [(i * 128, min(128, ROWS - i * 128)) for i in range((ROWS + 127) // 128)]
NCH = len(CHUNKS)
LAM2 = 0.1               # 2 * LAMBDA_OCC
RNE = float(2 ** 23)

# channel order grouped by e = dx % 3 (must match nothing host-side; any
# consistent permutation works since all downstream ops reduce over channels)
E_GROUPS = [(e, [dx for dx in range(7) if dx % 3 == e]) for e in range(3)]
CNT_E = [171, 171, 170]  # compacted column count per e


def _ts(eng, out, in0, s1, op0, s2=None, op1=None):
    return eng.tensor_scalar(out, in0, s1, s2,
                             op0, op1 if op1 is not None else ALU.bypass)


def build_program(debug_taps=False):
    tile_utils.max_sbuf_usage = 204 * 1024
    nc = bacc.Bacc("TRN2", target_bir_lowering=False, debug=False,
                   num_devices=NCORES)
    dbg = {}
    if debug_taps:
        dbg = {
            "d_gidx": nc.dram_tensor("d_gidx", [64, 400], F32, kind="ExternalOutput"),
            "d_feat": nc.dram_tensor("d_feat", [64, 7200], F32, kind="ExternalOutput"),
            "d_F": nc.dram_tensor("d_F", [64, 7200], F32, kind="ExternalOutput"),
            "d_s0": nc.dram_tensor("d_s0", [128, Q], F32, kind="ExternalOutput"),
            "d_idx": nc.dram_tensor("d_idx", [128, 16], F32, kind="ExternalOutput"),
            "d_cnt": nc.dram_tensor("d_cnt", [128, 50], F32, kind="ExternalOutput"),
            "d_sc": nc.dram_tensor("d_sc", [128, 96], F32, kind="ExternalOutput"),
            "d_ym": nc.dram_tensor("d_ym", [64, 1], F32, kind="ExternalOutput"),
            "d_n2": nc.dram_tensor("d_n2", [1, 7296], F32, kind="ExternalOutput"),
            "d_rs": nc.dram_tensor("d_rs", [128, 57], F32, kind="ExternalOutput"),
        }

    timg = nc.dram_tensor("timg", [512, 512], F32, kind="ExternalInput")
    rimg = nc.dram_tensor("rimg", [512, 512], F32, kind="ExternalInput")
    tfield = nc.dram_tensor("tfield", [ROWS, 2], F32, kind="ExternalInput")
    rfield = nc.dram_tensor("rfield", [ROWS, 2], F32, kind="ExternalInput")
    rg_in = nc.dram_tensor("rg_in", [C * ROWS], F32)
    rg_out = nc.dram_tensor("rg_out", [NCORES * C * ROWS], F32,
                            addr_space="Shared")
    out_t = nc.dram_tensor("out", [1, 1], F32, kind="ExternalOutput")

    ghbm = {n: nc.dram_tensor(f"ghbm_{n}", [3 * GRP], F32) for n in "tr"}
    img_pad = {n: nc.dram_tensor(f"imgpad_{n}", [520 * 512], F32) for n in "tr"}
    cc_in = nc.dram_tensor("cc_in", [Q], F32)
    cc_out = nc.dram_tensor("cc_out", [Q], F32, addr_space="Shared")
    ls_in = nc.dram_tensor("ls_in", [1], F32)
    ls_out = nc.dram_tensor("ls_out", [1], F32, addr_space="Shared")
    groups = [list(range(NCORES))]

    with tile.TileContext(nc) as tc:
        with tc.tile_pool(name="persist", bufs=1) as pp:
            F = pp.tile([64, 800 + Q], BF16, tag="F")        # xf | yf (+aug rows)
            iotaA = pp.tile([128, 50], F32, tag="iA")
            iotaB = pp.tile([128, 128], F32, tag="iB")
            ones_r = pp.tile([1, 64], F32, tag="ones_r")     # K=1 bcast lhsT (f32)
            ones_n2 = pp.tile([64, 1], BF16, tag="ones_n2")  # n2 lhsT (bf16)
            ones_c = pp.tile([128, 1], F32, tag="ones_c")
            loss_acc = pp.tile([128, 1], F32, tag="lacc")

            i32tmp = pp.tile([128, 128], I32, tag="i32tmp")
            nc.gpsimd.iota(i32tmp[:, 0:50], pattern=[[1, 50]], channel_multiplier=0)
            nc.vector.tensor_copy(iotaA[:], i32tmp[:, 0:50])
            nc.gpsimd.iota(i32tmp[:], pattern=[[1, 128]], channel_multiplier=0)
            nc.vector.tensor_copy(iotaB[:], i32tmp[:])
            nc.vector.memset(ones_r[:], 1.0)
            nc.vector.memset(ones_n2[:], 1.0)
            nc.vector.memset(ones_c[:], 1.0)
            nc.vector.memset(loss_acc[:], 0.0)
            # aug rows: zero rows 32-63 (rows 32-48 later overwritten by xf/yf
            # write), then ones into xf-aug via DMA (DVE can't start at p=49)
            nc.vector.memset(F[32:64, :], 0.0)
            aug2 = pp.tile([2, 800], BF16, tag="aug2")
            nc.vector.memset(aug2[:], 1.0)
            nc.sync.dma_start(F[49:51, 0:800], aug2[:])

            # =========== prologue: gather + normalize ====================
            with tc.tile_pool(name="pro", bufs=1) as pro:
                featcat = pro.tile([64, 800 + Q], F32, tag="featcat")

                with tc.tile_pool(name="progA", bufs=1) as gp:
                    apt = gp.tile([64, NPOS], F32, tag="apt")
                    nc.gpsimd.memset(apt[32:64, :], 0.0)

                    # ---- grid-sample indices (target shard + refer shard) ----
                    gidx = {}
                    for name, fld in (("t", tfield), ("r", rfield)):
                        nw = ROWS // 16
                        wx = gp.tile([16, ROWS // 16], F32, tag="wx")
                        wy = gp.tile([16, ROWS // 16], F32, tag="wy")
                        fv = fld[:].rearrange("(j p) c -> p j c", p=16)
                        nc.sync.dma_start(wx[:], fv[:, :, 0:1].squeeze(2))
                        nc.sync.dma_start(wy[:], fv[:, :, 1:2].squeeze(2))
                        for w in (wx, wy):
                            u = w[:, 0:nw]
                            _ts(nc.vector, u, u, 2.0, ALU.mult, 1.0, ALU.subtract)
                            _ts(nc.vector, u, u, 1.0, ALU.add, float(HO), ALU.mult)
                            _ts(nc.vector, u, u, 1.0, ALU.subtract, 0.5, ALU.mult)
                            _ts(nc.vector, u, u, RNE, ALU.add)
                            _ts(nc.vector, u, u, RNE, ALU.subtract)
                            _ts(nc.vector, u, u, 0.0, ALU.max, float(HO - 1), ALU.min)
                        nc.vector.scalar_tensor_tensor(
                            wy[:, 0:nw], wy[:, 0:nw], 171.0, wx[:, 0:nw],
                            ALU.mult, ALU.add)
                        if dbg and name == "r":
                            nc.sync.dma_start(dbg["d_gidx"][0:16, 0:nw], wy[:, 0:nw])
                        gi = gp.tile([64, ROWS // 16], I16, tag=f"gi_{name}")
                        nc.vector.tensor_copy(gi[0:16, 0:nw], wy[:, 0:nw])
                        for k in range(1, 4):
                            nc.sync.dma_start(gi[16 * k:16 * k + 16, 0:nw],
                                              gi[0:16, 0:nw])
                        gidx[name] = gi

                    # channel order: (e, rho, dyq, dxq)
                    rsh = gp.tile([64, ROWS], F32, tag="rsh")
                    qi = 0
                    for iname in ("r", "t"):
                        img = rimg if iname == "r" else timg
                        gh = ghbm[iname]
                        ipad = img_pad[iname]
                        # pad image to 520 rows in HBM
                        nc.sync.dma_start(
                            ipad[0:512 * 512].unsqueeze(0),
                            img[:].rearrange("a b -> (a b)").unsqueeze(0))
                        nc.scalar.dma_start(
                            ipad[512 * 512:520 * 512].unsqueeze(0),
                            img[:].rearrange("a b -> (a b)")[0:8 * 512].unsqueeze(0))
                        # residue-grouped staging: partitions (m,q) hold rows
                        # 3*(4q+k)+m (k=0..3), m in {0,1} in stg01, m=2 in stg2
                        stg01 = gp.tile([86, 4, 512], F32, tag="stg01")
                        stg2 = gp.tile([43, 4, 512], F32, tag="stg2")
                        for m in range(3):
                            srcap = bass.AP(tensor=ipad, offset=m * 512,
                                            ap=[[12 * 512, 43], [3 * 512, 4],
                                                [1, 512]])
                            dstt = stg01[43 * m:43 * m + 43] if m < 2 else stg2[:]
                            eng = nc.sync if qi % 2 == 0 else nc.scalar
                            eng.dma_start(dstt, srcap)
                            qi += 1
                        # per e: column-compact (DVE) then grouped write to HBM
                        # gh layout: [e-interleaved? no]: gh holds for each e a
                        # region of 3*GRP/..; we use separate offsets per e in
                        # THREE dram tensors? simpler: gh covers ONE e at a time
                        # is wrong -- instead allocate gh as [3e][3m][GRP] via
                        # offset e*3*GRP... but gh was sized [3*GRP]. We only
                        # need e's data until its apt DMA is done; process e
                        # sequentially reusing gh regions [m*GRP].
                        pbase = 0
                        for e, dxs in E_GROUPS:
                            cnt = CNT_E[e]
                            ce01 = gp.tile([86, 4, 171], F32, tag="ce01")
                            ce2 = gp.tile([43, 4, 171], F32, tag="ce2")
                            if cnt < 171:
                                nc.vector.memset(ce01[:, :, cnt:171], 0.0)
                                nc.vector.memset(ce2[:, :, cnt:171], 0.0)
                            nc.vector.tensor_copy(
                                ce01[:, :, 0:cnt],
                                stg01[:, :, e:e + 3 * (cnt - 1) + 1:3])
                            nc.vector.tensor_copy(
                                ce2[:, :, 0:cnt],
                                stg2[:, :, e:e + 3 * (cnt - 1) + 1:3])
                            for m in range(3):
                                srct = (ce01[43 * m:43 * m + 43] if m < 2
                                        else ce2[:])
                                dstap = bass.AP(tensor=gh, offset=m * GRP,
                                                ap=[[684, 43], [1, 684]])
                                eng = nc.sync if qi % 2 == 0 else nc.scalar
                                eng.dma_start(dstap, srct)
                                qi += 1
                            # table slices: channels (rho, dyq, dxq) for this e
                            for rho in range(3):
                                dys = [dy for dy in range(7) if dy % 3 == rho]
                                for dyq in range(len(dys)):
                                    srcap = bass.AP(
                                        tensor=gh,
                                        offset=rho * GRP + dyq * 171,
                                        ap=[[1, len(dxs)], [1, NPOS]])
                                    eng = nc.sync if qi % 2 == 0 else nc.scalar
                                    eng.dma_start(
                                        apt[pbase:pbase + len(dxs), :], srcap)
                                    qi += 1
                                    pbase += len(dxs)
                        assert pbase == C
                        # gather this image's shard of samples
                        gi = gidx[iname]
                        outt = (rsh[:, :] if iname == "r"
                                else featcat[:, 0:ROWS])
                        nc.gpsimd.ap_gather(
                            out_ap=outt.unsqueeze(2),
                            in_ap=apt[:].unsqueeze(2),
                            idxs_ap=gi[:, 0:ROWS // 16],
                            channels=64, num_elems=NPOS, d=1, num_idxs=ROWS)

                    # ---- all-gather refer features across cores ----
                    nc.sync.dma_start(
                        rg_in[:].rearrange("(c s) -> c s", c=C), rsh[0:C, :])
                    nc.gpsimd.collective_compute(
                        "AllGather", ALU.bypass, replica_groups=groups,
                        ins=[rg_in[:].opt()], outs=[rg_out[:].opt()])
                    srcap = bass.AP(tensor=rg_out, offset=0,
                                    ap=[[ROWS, C], [C * ROWS, NCORES],
                                        [1, ROWS]])
                    dstap = featcat[0:C, 800:800 + Q].rearrange(
                        "c (k s) -> c k s", k=NCORES)
                    nc.sync.dma_start(dstap, srcap)

                # ---- y_mean (refer), centered features ----
                if dbg:
                    nc.sync.dma_start(dbg["d_feat"][:], featcat[:])
                _gpcm = tc.tile_pool(name="progB", bufs=1)
                gp = _gpcm.__enter__()
                ymean = gp.tile([64, 1], F32, tag="ymean")
                nc.vector.memset(ymean[:], 0.0)
                nc.scalar.activation(featcat[0:C, 800:800 + Q],
                                     featcat[0:C, 800:800 + Q],
                                     ACTF.Copy, accum_out=ymean[0:C, :])
                if dbg:
                    nc.sync.dma_start(dbg["d_ym"][:], ymean[:])
                _ts(nc.vector, ymean[0:C, :], ymean[0:C, :], 1.0 / Q, ALU.mult)
                _ts(nc.vector, featcat[0:C, :], featcat[0:C, :],
                    ymean[0:C, :], ALU.subtract)

                # ---- column norms ----
                NT = 800 + Q
                sq = gp.tile([64, NT], BF16, tag="sq")
                nc.scalar.activation(sq[0:C, :], featcat[0:C, :], ACTF.Square)
                n2row = gp.tile([1, 7296], F32, tag="n2row")
                nc.vector.memset(n2row[:], 1.0)
                nsl = [(i * 512, min(512, NT - i * 512))
                       for i in range((NT + 511) // 512)]
                with tc.tile_pool(name="n2ps", bufs=2, space="PSUM") as n2p:
                    for r0 in range(0, len(nsl), 4):
                        sls = nsl[r0:r0 + 4]
                        ps = n2p.tile([1, 2048], F32, tag="n2psum")
                        for j, (o, w) in enumerate(sls):
                            nc.tensor.matmul(ps[:, j * 512:j * 512 + w],
                                             ones_n2[0:C, :], sq[0:C, o:o + w],
                                             start=True, stop=True)
                        o0 = sls[0][0]
                        wtot = sum(w for _, w in sls)
                        nc.scalar.activation(n2row[:, o0:o0 + wtot],
                                             ps[:, 0:wtot], ACTF.Copy)
                if dbg:
                    nc.sync.dma_start(dbg["d_n2"][:], n2row[:])
                # compact rsqrt with one Newton step
                cpt = gp.tile([128, 57], F32, tag="cpt")
                nc.sync.dma_start(
                    cpt[:], n2row[:].rearrange("a (p j) -> a p j", p=128))
                rc = gp.tile([128, 57], F32, tag="rc")
                nc.vector.reciprocal(rc[:], cpt[:])
                rs = gp.tile([128, 57], F32, tag="rs")
                nc.scalar.activation(rs[:], rc[:], ACTF.Sqrt)
                t2 = gp.tile([128, 57], F32, tag="t2")
                nc.vector.tensor_mul(t2[:], rs[:], rs[:])
                nc.vector.tensor_mul(t2[:], t2[:], cpt[:])
                _ts(nc.vector, t2[:], t2[:], -0.5, ALU.mult, 1.5, ALU.add)
                nc.vector.tensor_mul(rs[:], rs[:], t2[:])
                if dbg:
                    nc.sync.dma_start(dbg["d_rs"][:], rs[:])
                nc.sync.dma_start(
                    n2row[:].rearrange("a (p j) -> a p j", p=128), rs[:])
                # broadcast 1/norm to C partitions; write xf/yf bf16
                with tc.tile_pool(name="bcps", bufs=2, space="PSUM") as bp:
                    for o in range(0, NT, 2048):
                        w = min(2048, NT - o)
                        ps = bp.tile([64, 2048], F32, tag="bc")
                        for j in range(0, w, 512):
                            wj = min(512, w - j)
                            nc.tensor.matmul(ps[0:C, j:j + wj], ones_r[0:1, 0:C],
                                             n2row[:, o + j:o + j + wj],
                                             start=True, stop=True)
                        nc.vector.tensor_mul(F[0:C, o:o + w],
                                             featcat[0:C, o:o + w], ps[0:C, 0:w])
                if dbg:
                    nc.gpsimd.dma_start(dbg["d_F"][:], F[:])
                _gpcm.__exit__(None, None, None)

            # =========== main phases =====================================
            with tc.tile_pool(name="mid", bufs=1) as mp_:
                ohA = mp_.tile([128, NCH * 50], BF16, tag="ohA")
                ohB = mp_.tile([128, NCH * 128], BF16, tag="ohB")
                val8 = mp_.tile([128, 8], F32, tag="val8")
                idx8 = mp_.tile([128, 8], U32, tag="idx8")
                small = mp_.tile([128, 96], F32, tag="small")
                nc.vector.memset(small[:], 0.0)

                # ---- phase 1: sim matmul, f32 store, exact argmax ----
                with tc.tile_pool(name="ph1", bufs=2) as s1pool, \
                     tc.tile_pool(name="ph1ps", bufs=2, space="PSUM") as p1p:
                    for ci, (c0, pc) in enumerate(CHUNKS):
                        s_sl = s1pool.tile([128, Q], F32, tag="schunk")
                        for o in range(0, Q, 2048):
                            w = min(2048, Q - o)
                            ps = p1p.tile([128, 2048], F32, tag="s1")
                            for j in range(0, w, 512):
                                wj = min(512, w - j)
                                nc.tensor.matmul(
                                    ps[0:pc, j:j + wj], F[0:C, c0:c0 + pc],
                                    F[0:C, 800 + o + j:800 + o + j + wj],
                                    start=True, stop=True)
                            nc.scalar.activation(s_sl[0:pc, o:o + w],
                                                 ps[0:pc, 0:w], ACTF.Copy)
                        if dbg and ci == 0:
                            nc.sync.dma_start(dbg["d_s0"][:], s_sl[:])
                        nc.vector.max(val8[0:pc, :], s_sl[0:pc, :])
                        nc.vector.max_index(idx8[0:pc, :], val8[0:pc, :],
                                            s_sl[0:pc, :])
                        qf = small[:, ci:ci + 1]
                        nc.vector.tensor_copy(qf[0:pc, :], idx8[0:pc, 0:1])
                        af = small[:, 8 + ci:9 + ci]
                        _ts(nc.vector, af[0:pc, :], qf[0:pc, :], 1.0 / 128.0,
                            ALU.mult, 63.5 / 128.0, ALU.subtract)
                        _ts(nc.vector, af[0:pc, :], af[0:pc, :], RNE, ALU.add)
                        _ts(nc.vector, af[0:pc, :], af[0:pc, :], RNE, ALU.subtract)
                        bf_ = small[:, 16 + ci:17 + ci]
                        nc.vector.scalar_tensor_tensor(
                            bf_[0:pc, :], af[0:pc, :], -128.0, qf[0:pc, :],
                            ALU.mult, ALU.add)
                        _ts(nc.vector, ohA[0:pc, ci * 50:(ci + 1) * 50],
                            iotaA[0:pc, :], af[0:pc, :], ALU.is_equal)
                        _ts(nc.vector, ohB[0:pc, ci * 128:(ci + 1) * 128],
                            iotaB[0:pc, :], bf_[0:pc, :], ALU.is_equal)

                # ---- phase 2: histogram, all-reduce, occ rows ----
                with tc.tile_pool(name="ph2", bufs=1) as hp, \
                     tc.tile_pool(name="ph2ps", bufs=1, space="PSUM") as cp:
                    cpsum = cp.tile([64, 128], F32, tag="cpsum")
                    for ci, (c0, pc) in enumerate(CHUNKS):
                        nc.tensor.matmul(cpsum[0:50, :],
                                         ohA[0:pc, ci * 50:(ci + 1) * 50],
                                         ohB[0:pc, ci * 128:(ci + 1) * 128],
                                         start=(ci == 0), stop=(ci == NCH - 1))
                    csb = hp.tile([64, 128], F32, tag="csb")
                    nc.vector.tensor_copy(csb[0:50, :], cpsum[0:50, :])
                    nc.sync.dma_start(
                        cc_in[:].rearrange("(p j) -> p j", p=50), csb[0:50, :])
                    nc.gpsimd.collective_compute(
                        "AllReduce", ALU.add, replica_groups=groups,
                        ins=[cc_in[:].opt()], outs=[cc_out[:].opt()])
                    ccp = hp.tile([128, 50], F32, tag="ccp")
                    nc.sync.dma_start(ccp[:],
                                      cc_out[:].rearrange("(p j) -> p j", p=128))
                    if dbg:
                        nc.sync.dma_start(dbg["d_cnt"][:], ccp[:])
                    th = hp.tile([128, 50], F32, tag="th")
                    _ts(nc.vector, th[:], ccp[:], -LAM2, ALU.mult)
                    hh = hp.tile([128, 50], BF16, tag="hh")
                    nc.vector.tensor_copy(hh[:], th[:])
                    ll = hp.tile([128, 50], BF16, tag="ll")
                    nc.vector.tensor_sub(ll[:], th[:], hh[:])
                    nc.sync.dma_start(F[49:50, 800:800 + Q], hh[:])
                    nc.sync.dma_start(F[50:51, 800:800 + Q], ll[:])

                # ---- phase 3: s' matmuls, row min, exp-sum, loss ----
                SPW = 1024
                nsub = (Q + SPW - 1) // SPW
                wdump = mp_.tile([128, SPW], BF16, tag="wdump")
                mparts = mp_.tile([128, nsub], F32, tag="mparts")
                sparts = mp_.tile([128, nsub], F32, tag="sparts")
                with tc.tile_pool(name="ph3a", bufs=2, space="PSUM") as pa, \
                     tc.tile_pool(name="ph3b", bufs=2, space="PSUM") as pb:
                    for ci, (c0, pc) in enumerate(CHUNKS):
                        for si in range(nsub):
                            o = si * SPW
                            w = min(SPW, Q - o)
                            ps = pa.tile([128, SPW], F32, tag="s3")
                            for j in range(0, w, 512):
                                wj = min(512, w - j)
                                nc.tensor.matmul(
                                    ps[0:pc, j:j + wj], F[0:KA, c0:c0 + pc],
                                    F[0:KA, 800 + o + j:800 + o + j + wj],
                                    start=True, stop=True)
                            nc.vector.tensor_reduce(
                                mparts[0:pc, si:si + 1], ps[0:pc, 0:w],
                                axis=AXX, op=ALU.max)
                        mxp = small[:, 24 + ci:25 + ci]
                        nc.vector.tensor_reduce(mxp[0:pc, :], mparts[0:pc, :],
                                                axis=AXX, op=ALU.max)
                        m_ = small[:, 32 + ci:33 + ci]
                        _ts(nc.vector, m_[0:pc, :], mxp[0:pc, :], -0.5, ALU.mult,
                            0.5, ALU.add)
                        a_ = small[:, 40 + ci:41 + ci]
                        nc.vector.reciprocal(a_[0:pc, :], m_[0:pc, :])
                        b_ = small[:, 48 + ci:49 + ci]
                        _ts(nc.vector, b_[0:pc, :], a_[0:pc, :], -1.0, ALU.mult,
                            2.0, ALU.add)
                        logm = small[:, 56 + ci:57 + ci]
                        nc.vector.scalar_tensor_tensor(
                            logm[0:pc, :], mxp[0:pc, :], a_[0:pc, :], b_[0:pc, :],
                            ALU.mult, ALU.add)
                        for si in range(nsub):
                            o = si * SPW
                            w = min(SPW, Q - o)
                            ps = pb.tile([128, SPW], F32, tag="s4")
                            for j in range(0, w, 512):
                                wj = min(512, w - j)
                                nc.tensor.matmul(
                                    ps[0:pc, j:j + wj], F[0:KA, c0:c0 + pc],
                                    F[0:KA, 800 + o + j:800 + o + j + wj],
                                    start=True, stop=True)
                            nc.scalar.activation(
                                wdump[0:pc, 0:w], ps[0:pc, 0:w], ACTF.Exp,
                                bias=b_[0:pc, :], scale=a_[0:pc, :],
                                accum_out=sparts[0:pc, si:si + 1])
                        S_ = small[:, 64 + ci:65 + ci]
                        nc.vector.tensor_reduce(S_[0:pc, :], sparts[0:pc, :],
                                                axis=AXX, op=ALU.add)
                        lnS = small[:, 72 + ci:73 + ci]
                        nc.scalar.activation(lnS[0:pc, :], S_[0:pc, :], ACTF.Ln)
                        lc = small[:, 80 + ci:81 + ci]
                        nc.vector.tensor_sub(lc[0:pc, :], lnS[0:pc, :],
                                             logm[0:pc, :])
                        nc.vector.tensor_add(loss_acc[0:pc, :],
                                             loss_acc[0:pc, :], lc[0:pc, :])

                # ---- final reduce + all-reduce ----
                with tc.tile_pool(name="fin", bufs=1) as fp, \
                     tc.tile_pool(name="finps", bufs=1, space="PSUM") as fps:
                    tot = fps.tile([1, 1], F32, tag="tot")
                    nc.tensor.matmul(tot[:], loss_acc[:], ones_c[:],
                                     start=True, stop=True)
                    tsb = fp.tile([1, 1], F32, tag="tsb")
                    nc.vector.tensor_copy(tsb[:], tot[:])
                    nc.sync.dma_start(ls_in[:].unsqueeze(0), tsb[:])
                    nc.gpsimd.collective_compute(
                        "AllReduce", ALU.add, replica_groups=groups,
                        ins=[ls_in[:].opt()], outs=[ls_out[:].opt()])
                    res = fp.tile([1, 1], F32, tag="res")
                    nc.sync.dma_start(res[:], ls_out[:].unsqueeze(0))
                    if dbg:
                        nc.sync.dma_start(dbg["d_sc"][:], small[:])
                    _ts(nc.vector, res[:], res[:], 1.0 / Q, ALU.mult)
                    nc.sync.dma_start(out_t[:], res[:])

    nc.compile()
    return nc


_NC = None


def _get_nc():
    global _NC
    if _NC is None:
        _NC = build_program()
    return _NC


def make_in_maps(target_features, refer_features, target_field, refer_field):
    timg_np = np.ascontiguousarray(
        np.asarray(target_features, np.float32).reshape(512, 512))
    rimg_np = np.ascontiguousarray(
        np.asarray(refer_features, np.float32).reshape(512, 512))
    tf = np.ascontiguousarray(np.asarray(target_field, np.float32).reshape(-1, 2))
    rf = np.ascontiguousarray(np.asarray(refer_field, np.float32).reshape(-1, 2))
    in_maps = []
    for k in range(NCORES):
        in_maps.append({
            "timg": timg_np,
            "rimg": rimg_np,
            "tfield": np.ascontiguousarray(tf[k * ROWS:(k + 1) * ROWS]),
            "rfield": np.ascontiguousarray(rf[k * ROWS:(k + 1) * ROWS]),
        })
    return in_maps


LAST_RESULTS = None


def kernel(target_features, refer_features, target_field, refer_field,
           args=None, **_ignored):
    global LAST_RESULTS
    from concourse import bass_utils
    nc = _get_nc()
    in_maps = make_in_maps(target_features, refer_features,
                           target_field, refer_field)
    res = bass_utils.run_bass_kernel_spmd(
        nc, in_maps, core_ids=list(range(NCORES)),
        trace=bool(int(os.environ.get("AGC_TRACE", "0"))))
    LAST_RESULTS = res
    return np.asarray(res.results[0]["out"], np.float32).reshape(())


if __name__ == "__main__":
    if "--build" in sys.argv:
        build_program()
        print("BUILD OK")


# revision 32
# speedup vs baseline: 1.2209x; 1.0488x over previous
"""AGC loss kernel for 8 Trainium2 NeuronCores (Bass/Tile).

Self-contained: builds the Bass program, shards inputs host-side, runs via
run_bass_kernel_spmd, returns the full (scalar) output.

Device mapping: target rows p sharded 800/core; sim = bf16 matmul with f32
PSUM; occurrence term folded into the matmul via two extra bf16 contraction
rows (hi/lo split); exact first-index argmax via max/max_index on f32 sim.
"""
import os
import sys
import numpy as np

for _p in ("/opt/trn_rl_repo", os.environ.get("TRN_RL_REPO", "")):
    if _p and _p not in sys.path and os.path.isdir(_p):
        sys.path.insert(0, _p)

import concourse.bass as bass
import concourse.mybir as mybir
from concourse import bacc, tile
from concourse import tile_utils

F32 = mybir.dt.float32
BF16 = mybir.dt.bfloat16
I16 = mybir.dt.int16
I32 = mybir.dt.int32
U32 = mybir.dt.uint32
ALU = mybir.AluOpType
ACTF = mybir.ActivationFunctionType
AXX = mybir.AxisListType.X

HO = 169                 # patch grid 169x169
GRP = 43 * 684           # residue-group stride in the grouped image (29412)
NPOS = 171 * 168 + 168 + 1   # 28897 table elems per channel (g = 171*iy + ix)
C = 49                   # channels (7x7 patch)
KA = 51                  # + 2 aug rows (occ hi/lo)
Q = 6400                 # refer samples (columns)
NCORES = 8
ROWS = Q // NCORES       # 800 target rows per core
CHUNKS = [(i * 128, min(128, ROWS - i * 128)) for i in range((ROWS + 127) // 128)]
NCH = len(CHUNKS)
LAM2 = 0.1               # 2 * LAMBDA_OCC
RNE = float(2 ** 23)

# channel order grouped by e = dx % 3 (must match nothing host-side; any
# consistent permutation works since all downstream ops reduce over channels)
E_GROUPS = [(e, [dx for dx in range(7) if dx % 3 == e]) for e in range(3)]
CNT_E = [171, 171, 170]  # compacted column count per e


def _ts(eng, out, in0, s1, op0, s2=None, op1=None):
    return eng.tensor_scalar(out, in0, s1, s2,
                             op0, op1 if op1 is not None else ALU.bypass)


def build_program(debug_taps=False):
    tile_utils.max_sbuf_usage = 204 * 1024
    nc = bacc.Bacc("TRN2", target_bir_lowering=False, debug=False,
                   num_devices=NCORES)
    dbg = {}
    if debug_taps:
        dbg = {
            "d_gidx": nc.dram_tensor("d_gidx", [64, 400], F32, kind="ExternalOutput"),
            "d_feat": nc.dram_tensor("d_feat", [64, 7200], F32, kind="ExternalOutput"),
            "d_F": nc.dram_tensor("d_F", [64, 7200], F32, kind="ExternalOutput"),
            "d_s0": nc.dram_tensor("d_s0", [128, Q], F32, kind="ExternalOutput"),
            "d_idx": nc.dram_tensor("d_idx", [128, 16], F32, kind="ExternalOutput"),
            "d_cnt": nc.dram_tensor("d_cnt", [128, 50], F32, kind="ExternalOutput"),
            "d_sc": nc.dram_tensor("d_sc", [128, 96], F32, kind="ExternalOutput"),
            "d_ym": nc.dram_tensor("d_ym", [64, 1], F32, kind="ExternalOutput"),
            "d_n2": nc.dram_tensor("d_n2", [1, 7296], F32, kind="ExternalOutput"),
            "d_rs": nc.dram_tensor("d_rs", [128, 57], F32, kind="ExternalOutput"),
        }

    timg = nc.dram_tensor("timg", [512, 512], F32, kind="ExternalInput")
    rimg = nc.dram_tensor("rimg", [512, 512], F32, kind="ExternalInput")
    tfield = nc.dram_tensor("tfield", [ROWS, 2], F32, kind="ExternalInput")
    rfield = nc.dram_tensor("rfield", [ROWS, 2], F32, kind="ExternalInput")
    rg_in = nc.dram_tensor("rg_in", [C * ROWS], F32)
    rg_out = nc.dram_tensor("rg_out", [NCORES * C * ROWS], F32,
                            addr_space="Shared")
    out_t = nc.dram_tensor("out", [1, 1], F32, kind="ExternalOutput")

    ghbm = {n: nc.dram_tensor(f"ghbm_{n}", [3 * GRP], BF16) for n in "tr"}
    sh_dram = {n: nc.dram_tensor(f"shd_{n}", [ROWS], F32) for n in "tr"}
    img_pad = {n: nc.dram_tensor(f"imgpad_{n}", [520 * 512], F32) for n in "tr"}
    cc_in = nc.dram_tensor("cc_in", [Q], F32)
    cc_out = nc.dram_tensor("cc_out", [Q], F32, addr_space="Shared")
    ls_in = nc.dram_tensor("ls_in", [1], F32)
    ls_out = nc.dram_tensor("ls_out", [1], F32, addr_space="Shared")
    groups = [list(range(NCORES))]

    with tile.TileContext(nc) as tc:
        with tc.tile_pool(name="persist", bufs=1) as pp:
            F = pp.tile([64, 800 + Q], BF16, tag="F")        # xf | yf (+aug rows)
            iotaA = pp.tile([128, 50], F32, tag="iA")
            iotaB = pp.tile([128, 128], F32, tag="iB")
            ones_r = pp.tile([1, 64], F32, tag="ones_r")     # K=1 bcast lhsT (f32)
            ones_n2 = pp.tile([64, 1], BF16, tag="ones_n2")  # n2 lhsT (bf16)
            ones_c = pp.tile([128, 1], F32, tag="ones_c")
            loss_acc = pp.tile([128, 1], F32, tag="lacc")

            i32tmp = pp.tile([128, 128], I32, tag="i32tmp")
            nc.gpsimd.iota(i32tmp[:, 0:50], pattern=[[1, 50]], channel_multiplier=0)
            nc.vector.tensor_copy(iotaA[:], i32tmp[:, 0:50])
            nc.gpsimd.iota(i32tmp[:], pattern=[[1, 128]], channel_multiplier=0)
            nc.vector.tensor_copy(iotaB[:], i32tmp[:])
            nc.vector.memset(ones_r[:], 1.0)
            nc.vector.memset(ones_n2[:], 1.0)
            nc.vector.memset(ones_c[:], 1.0)
            nc.vector.memset(loss_acc[:], 0.0)
            # aug rows: zero rows 32-63 (rows 32-48 later overwritten by xf/yf
            # write), then ones into xf-aug via DMA (DVE can't start at p=49)
            nc.vector.memset(F[32:64, :], 0.0)
            aug2 = pp.tile([2, 800], BF16, tag="aug2")
            nc.vector.memset(aug2[:], 1.0)
            nc.sync.dma_start(F[49:51, 0:800], aug2[:])

            # =========== prologue: gather + normalize ====================
            with tc.tile_pool(name="pro", bufs=1) as pro:
                featcat = pro.tile([64, 800 + Q], F32, tag="featcat")

                with tc.tile_pool(name="progA", bufs=1) as gp:
                    NPOSW = (NPOS + 3) // 2  # u32 words (pad to even+1)
                    apt = gp.tile([64, 2 * NPOSW], BF16, tag="apt")
                    nc.gpsimd.memset(apt[32:64, :], 0.0)

                    # ---- grid-sample indices (target shard + refer shard) ----
                    gidx = {}
                    for name, fld in (("t", tfield), ("r", rfield)):
                        nw = ROWS // 16
                        wx = gp.tile([16, ROWS // 16], F32, tag="wx")
                        wy = gp.tile([16, ROWS // 16], F32, tag="wy")
                        fv = fld[:].rearrange("(j p) c -> p j c", p=16)
                        nc.sync.dma_start(wx[:], fv[:, :, 0:1].squeeze(2))
                        nc.sync.dma_start(wy[:], fv[:, :, 1:2].squeeze(2))
                        for w in (wx, wy):
                            u = w[:, 0:nw]
                            _ts(nc.vector, u, u, 2.0, ALU.mult, 1.0, ALU.subtract)
                            _ts(nc.vector, u, u, 1.0, ALU.add, float(HO), ALU.mult)
                            _ts(nc.vector, u, u, 1.0, ALU.subtract, 0.5, ALU.mult)
                            _ts(nc.vector, u, u, RNE, ALU.add)
                            _ts(nc.vector, u, u, RNE, ALU.subtract)
                            _ts(nc.vector, u, u, 0.0, ALU.max, float(HO - 1), ALU.min)
                        nc.vector.scalar_tensor_tensor(
                            wy[:, 0:nw], wy[:, 0:nw], 171.0, wx[:, 0:nw],
                            ALU.mult, ALU.add)
                        if dbg and name == "r":
                            nc.sync.dma_start(dbg["d_gidx"][0:16, 0:nw], wy[:, 0:nw])
                        # word index j = floor(g/2)
                        wj = gp.tile([16, ROWS // 16], F32, tag="wjf")
                        _ts(nc.vector, wj[:], wy[:, 0:nw], 0.5, ALU.mult, 0.25,
                            ALU.subtract)
                        _ts(nc.vector, wj[:], wj[:], RNE, ALU.add)
                        _ts(nc.vector, wj[:], wj[:], RNE, ALU.subtract)
                        gi = gp.tile([64, ROWS // 16], I16, tag=f"gi_{name}")
                        nc.vector.tensor_copy(gi[0:16, 0:nw], wj[:])
                        for k in range(1, 4):
                            nc.sync.dma_start(gi[16 * k:16 * k + 16, 0:nw],
                                              gi[0:16, 0:nw])
                        # shamt = 16*(1-parity) in partition-major [50,16]
                        # layout (s = 16p + j) for a contiguous DRAM riffle
                        px = gp.tile([50, 16], F32, tag="px")
                        py = gp.tile([50, 16], F32, tag="py")
                        fv2 = fld[:].rearrange("(p j) c -> p j c", p=50)
                        nc.sync.dma_start(px[:], fv2[:, :, 0:1].squeeze(2))
                        nc.sync.dma_start(py[:], fv2[:, :, 1:2].squeeze(2))
                        for w in (px, py):
                            u = w[:]
                            _ts(nc.vector, u, u, 2.0, ALU.mult, 1.0, ALU.subtract)
                            _ts(nc.vector, u, u, 1.0, ALU.add, float(HO), ALU.mult)
                            _ts(nc.vector, u, u, 1.0, ALU.subtract, 0.5, ALU.mult)
                            _ts(nc.vector, u, u, RNE, ALU.add)
                            _ts(nc.vector, u, u, RNE, ALU.subtract)
                            _ts(nc.vector, u, u, 0.0, ALU.max, float(HO - 1), ALU.min)
                        nc.vector.scalar_tensor_tensor(
                            py[:], py[:], 171.0, px[:], ALU.mult, ALU.add)
                        pj = gp.tile([50, 16], F32, tag="pj")
                        _ts(nc.vector, pj[:], py[:], 0.5, ALU.mult, 0.25,
                            ALU.subtract)
                        _ts(nc.vector, pj[:], pj[:], RNE, ALU.add)
                        _ts(nc.vector, pj[:], pj[:], RNE, ALU.subtract)
                        # shamt = 16 + 16*(2j - g)
                        _ts(nc.vector, pj[:], pj[:], 2.0, ALU.mult)
                        nc.vector.tensor_sub(pj[:], pj[:], py[:])
                        _ts(nc.vector, pj[:], pj[:], 16.0, ALU.mult, 16.0, ALU.add)
                        nc.sync.dma_start(
                            sh_dram[name][:].rearrange("(p j) -> p j", p=50),
                            pj[:])
                        shrow = gp.tile([1, ROWS], F32, tag=f"shrow_{name}")
                        nc.sync.dma_start(shrow[:],
                                          sh_dram[name][:].unsqueeze(0))
                        gidx[name] = (gi, shrow)

                    # channel order: (e, rho, dyq, dxq)
                    rsh = gp.tile([64, ROWS], F32, tag="rsh")
                    qi = 0
                    for iname in ("r", "t"):
                        img = rimg if iname == "r" else timg
                        gh = ghbm[iname]
                        ipad = img_pad[iname]
                        # pad image to 520 rows in HBM
                        nc.sync.dma_start(
                            ipad[0:512 * 512].unsqueeze(0),
                            img[:].rearrange("a b -> (a b)").unsqueeze(0))
                        nc.scalar.dma_start(
                            ipad[512 * 512:520 * 512].unsqueeze(0),
                            img[:].rearrange("a b -> (a b)")[0:8 * 512].unsqueeze(0))
                        # residue-grouped staging: partitions (m,q) hold rows
                        # 3*(4q+k)+m (k=0..3), m in {0,1} in stg01, m=2 in stg2
                        stg01 = gp.tile([86, 4, 512], F32, tag="stg01")
                        stg2 = gp.tile([43, 4, 512], F32, tag="stg2")
                        for m in range(3):
                            srcap = bass.AP(tensor=ipad, offset=m * 512,
                                            ap=[[12 * 512, 43], [3 * 512, 4],
                                                [1, 512]])
                            dstt = stg01[43 * m:43 * m + 43] if m < 2 else stg2[:]
                            eng = nc.sync if qi % 2 == 0 else nc.scalar
                            eng.dma_start(dstt, srcap)
                            qi += 1
                        # per e: column-compact (DVE) then grouped write to HBM
                        # gh layout: [e-interleaved? no]: gh holds for each e a
                        # region of 3*GRP/..; we use separate offsets per e in
                        # THREE dram tensors? simpler: gh covers ONE e at a time
                        # is wrong -- instead allocate gh as [3e][3m][GRP] via
                        # offset e*3*GRP... but gh was sized [3*GRP]. We only
                        # need e's data until its apt DMA is done; process e
                        # sequentially reusing gh regions [m*GRP].
                        pbase = 0
                        for e, dxs in E_GROUPS:
                            cnt = CNT_E[e]
                            ce01 = gp.tile([86, 4, 171], BF16, tag="ce01")
                            ce2 = gp.tile([43, 4, 171], BF16, tag="ce2")
                            if cnt < 171:
                                nc.vector.memset(ce01[:, :, cnt:171], 0.0)
                                nc.vector.memset(ce2[:, :, cnt:171], 0.0)
                            nc.vector.tensor_copy(
                                ce01[:, :, 0:cnt],
                                stg01[:, :, e:e + 3 * (cnt - 1) + 1:3])
                            nc.vector.tensor_copy(
                                ce2[:, :, 0:cnt],
                                stg2[:, :, e:e + 3 * (cnt - 1) + 1:3])
                            for m in range(3):
                                srct = (ce01[43 * m:43 * m + 43] if m < 2
                                        else ce2[:])
                                dstap = bass.AP(tensor=gh, offset=m * GRP,
                                                ap=[[684, 43], [1, 684]])
                                eng = nc.sync if qi % 2 == 0 else nc.scalar
                                eng.dma_start(dstap, srct)
                                qi += 1
                            # table slices: channels (rho, dyq, dxq) for this e
                            for rho in range(3):
                                dys = [dy for dy in range(7) if dy % 3 == rho]
                                nch = len(dys) * len(dxs)
                                srcap = bass.AP(
                                    tensor=gh,
                                    offset=rho * GRP,
                                    ap=[[171, len(dys)], [1, len(dxs)],
                                        [1, 2 * NPOSW]])
                                eng = nc.sync if qi % 2 == 0 else nc.scalar
                                eng.dma_start(
                                    apt[pbase:pbase + nch, 0:2 * NPOSW], srcap)
                                qi += 1
                                pbase += nch
                        assert pbase == C
                        # gather this image's shard of samples (u32 pairs)
                        gi, shrow = gidx[iname]
                        outt = (rsh[:, :] if iname == "r"
                                else featcat[:, 0:ROWS])
                        gout = gp.tile([64, ROWS], I32, tag="gout")
                        nc.gpsimd.ap_gather(
                            out_ap=gout[:].unsqueeze(2),
                            in_ap=apt[:].bitcast(I32).unsqueeze(2),
                            idxs_ap=gi[:, 0:ROWS // 16],
                            channels=64, num_elems=NPOSW, d=1, num_idxs=ROWS)
                        # replicate shamt row to 64 partitions via K=1 matmul
                        with tc.tile_pool(name="shps", bufs=1,
                                          space="PSUM") as shp:
                            shps = shp.tile([64, ROWS], F32, tag="shps")
                            for j0 in range(0, ROWS, 512):
                                wj0 = min(512, ROWS - j0)
                                nc.tensor.matmul(shps[:, j0:j0 + wj0],
                                                 ones_r[0:1, :],
                                                 shrow[:, j0:j0 + wj0],
                                                 start=True, stop=True)
                            shi = gp.tile([64, ROWS], I32, tag="shi")
                            nc.vector.tensor_copy(shi[:], shps[:])
                        nc.vector.tensor_tensor(
                            gout[:], gout[:], shi[:],
                            op=ALU.logical_shift_left)
                        _ts(nc.vector, gout[:], gout[:], -65536,
                            ALU.bitwise_and)
                        nc.vector.tensor_copy(outt, gout[:].bitcast(F32))

                    # ---- all-gather refer features across cores ----
                    nc.sync.dma_start(
                        rg_in[:].rearrange("(c s) -> c s", c=C), rsh[0:C, :])
                    nc.gpsimd.collective_compute(
                        "AllGather", ALU.bypass, replica_groups=groups,
                        ins=[rg_in[:].opt()], outs=[rg_out[:].opt()])
                    srcap = bass.AP(tensor=rg_out, offset=0,
                                    ap=[[ROWS, C], [C * ROWS, NCORES],
                                        [1, ROWS]])
                    dstap = featcat[0:C, 800:800 + Q].rearrange(
                        "c (k s) -> c k s", k=NCORES)
                    nc.sync.dma_start(dstap, srcap)

                # ---- y_mean (refer), centered features ----
                if dbg:
                    nc.sync.dma_start(dbg["d_feat"][:], featcat[:])
                _gpcm = tc.tile_pool(name="progB", bufs=1)
                gp = _gpcm.__enter__()
                ymean = gp.tile([64, 1], F32, tag="ymean")
                nc.vector.memset(ymean[:], 0.0)
                nc.scalar.activation(featcat[0:C, 800:800 + Q],
                                     featcat[0:C, 800:800 + Q],
                                     ACTF.Copy, accum_out=ymean[0:C, :])
                if dbg:
                    nc.sync.dma_start(dbg["d_ym"][:], ymean[:])
                _ts(nc.vector, ymean[0:C, :], ymean[0:C, :], 1.0 / Q, ALU.mult)
                _ts(nc.vector, featcat[0:C, :], featcat[0:C, :],
                    ymean[0:C, :], ALU.subtract)

                # ---- column norms ----
                NT = 800 + Q
                sq = gp.tile([64, NT], BF16, tag="sq")
                nc.scalar.activation(sq[0:C, :], featcat[0:C, :], ACTF.Square)
                n2row = gp.tile([1, 7296], F32, tag="n2row")
                nc.vector.memset(n2row[:], 1.0)
                nsl = [(i * 512, min(512, NT - i * 512))
                       for i in range((NT + 511) // 512)]
                with tc.tile_pool(name="n2ps", bufs=2, space="PSUM") as n2p:
                    for r0 in range(0, len(nsl), 4):
                        sls = nsl[r0:r0 + 4]
                        ps = n2p.tile([1, 2048], F32, tag="n2psum")
                        for j, (o, w) in enumerate(sls):
                            nc.tensor.matmul(ps[:, j * 512:j * 512 + w],
                                             ones_n2[0:C, :], sq[0:C, o:o + w],
                                             start=True, stop=True)
                        o0 = sls[0][0]
                        wtot = sum(w for _, w in sls)
                        nc.scalar.activation(n2row[:, o0:o0 + wtot],
                                             ps[:, 0:wtot], ACTF.Copy)
                if dbg:
                    nc.sync.dma_start(dbg["d_n2"][:], n2row[:])
                # compact rsqrt with one Newton step
                cpt = gp.tile([128, 57], F32, tag="cpt")
                nc.sync.dma_start(
                    cpt[:], n2row[:].rearrange("a (p j) -> a p j", p=128))
                rc = gp.tile([128, 57], F32, tag="rc")
                nc.vector.reciprocal(rc[:], cpt[:])
                rs = gp.tile([128, 57], F32, tag="rs")
                nc.scalar.activation(rs[:], rc[:], ACTF.Sqrt)
                t2 = gp.tile([128, 57], F32, tag="t2")
                nc.vector.tensor_mul(t2[:], rs[:], rs[:])
                nc.vector.tensor_mul(t2[:], t2[:], cpt[:])
                _ts(nc.vector, t2[:], t2[:], -0.5, ALU.mult, 1.5, ALU.add)
                nc.vector.tensor_mul(rs[:], rs[:], t2[:])
                if dbg:
                    nc.sync.dma_start(dbg["d_rs"][:], rs[:])
                nc.sync.dma_start(
                    n2row[:].rearrange("a (p j) -> a p j", p=128), rs[:])
                # broadcast 1/norm to C partitions; write xf/yf bf16
                with tc.tile_pool(name="bcps", bufs=2, space="PSUM") as bp:
                    for o in range(0, NT, 2048):
                        w = min(2048, NT - o)
                        ps = bp.tile([64, 2048], F32, tag="bc")
                        for j in range(0, w, 512):
                            wj = min(512, w - j)
                            nc.tensor.matmul(ps[0:C, j:j + wj], ones_r[0:1, 0:C],
                                             n2row[:, o + j:o + j + wj],
                                             start=True, stop=True)
                        nc.vector.tensor_mul(F[0:C, o:o + w],
                                             featcat[0:C, o:o + w], ps[0:C, 0:w])
                if dbg:
                    nc.gpsimd.dma_start(dbg["d_F"][:], F[:])
                _gpcm.__exit__(None, None, None)

            # =========== main phases =====================================
            with tc.tile_pool(name="mid", bufs=1) as mp_:
                ohA = mp_.tile([128, NCH * 50], BF16, tag="ohA")
                ohB = mp_.tile([128, NCH * 128], BF16, tag="ohB")
                val8 = mp_.tile([128, 8], F32, tag="val8")
                idx8 = mp_.tile([128, 8], U32, tag="idx8")
                small = mp_.tile([128, 96], F32, tag="small")
                nc.vector.memset(small[:], 0.0)

                # ---- phase 1: sim matmul, f32 store, exact argmax ----
                with tc.tile_pool(name="ph1", bufs=2) as s1pool, \
                     tc.tile_pool(name="ph1ps", bufs=2, space="PSUM") as p1p:
                    for ci, (c0, pc) in enumerate(CHUNKS):
                        s_sl = s1pool.tile([128, Q], F32, tag="schunk")
                        for o in range(0, Q, 2048):
                            w = min(2048, Q - o)
                            ps = p1p.tile([128, 2048], F32, tag="s1")
                            for j in range(0, w, 512):
                                wj = min(512, w - j)
                                nc.tensor.matmul(
                                    ps[0:pc, j:j + wj], F[0:C, c0:c0 + pc],
                                    F[0:C, 800 + o + j:800 + o + j + wj],
                                    start=True, stop=True)
                            nc.scalar.activation(s_sl[0:pc, o:o + w],
                                                 ps[0:pc, 0:w], ACTF.Copy)
                        if dbg and ci == 0:
                            nc.sync.dma_start(dbg["d_s0"][:], s_sl[:])
                        nc.vector.max(val8[0:pc, :], s_sl[0:pc, :])
                        nc.vector.max_index(idx8[0:pc, :], val8[0:pc, :],
                                            s_sl[0:pc, :])
                        qf = small[:, ci:ci + 1]
                        nc.vector.tensor_copy(qf[0:pc, :], idx8[0:pc, 0:1])
                        af = small[:, 8 + ci:9 + ci]
                        _ts(nc.vector, af[0:pc, :], qf[0:pc, :], 1.0 / 128.0,
                            ALU.mult, 63.5 / 128.0, ALU.subtract)
                        _ts(nc.vector, af[0:pc, :], af[0:pc, :], RNE, ALU.add)
                        _ts(nc.vector, af[0:pc, :], af[0:pc, :], RNE, ALU.subtract)
                        bf_ = small[:, 16 + ci:17 + ci]
                        nc.vector.scalar_tensor_tensor(
                            bf_[0:pc, :], af[0:pc, :], -128.0, qf[0:pc, :],
                            ALU.mult, ALU.add)
                        _ts(nc.vector, ohA[0:pc, ci * 50:(ci + 1) * 50],
                            iotaA[0:pc, :], af[0:pc, :], ALU.is_equal)
                        _ts(nc.vector, ohB[0:pc, ci * 128:(ci + 1) * 128],
                            iotaB[0:pc, :], bf_[0:pc, :], ALU.is_equal)

                # ---- phase 2: histogram, all-reduce, occ rows ----
                with tc.tile_pool(name="ph2", bufs=1) as hp, \
                     tc.tile_pool(name="ph2ps", bufs=1, space="PSUM") as cp:
                    cpsum = cp.tile([64, 128], F32, tag="cpsum")
                    for ci, (c0, pc) in enumerate(CHUNKS):
                        nc.tensor.matmul(cpsum[0:50, :],
                                         ohA[0:pc, ci * 50:(ci + 1) * 50],
                                         ohB[0:pc, ci * 128:(ci + 1) * 128],
                                         start=(ci == 0), stop=(ci == NCH - 1))
                    csb = hp.tile([64, 128], F32, tag="csb")
                    nc.vector.tensor_copy(csb[0:50, :], cpsum[0:50, :])
                    nc.sync.dma_start(
                        cc_in[:].rearrange("(p j) -> p j", p=50), csb[0:50, :])
                    nc.gpsimd.collective_compute(
                        "AllReduce", ALU.add, replica_groups=groups,
                        ins=[cc_in[:].opt()], outs=[cc_out[:].opt()])
                    ccp = hp.tile([128, 50], F32, tag="ccp")
                    nc.sync.dma_start(ccp[:],
                                      cc_out[:].rearrange("(p j) -> p j", p=128))
                    if dbg:
                        nc.sync.dma_start(dbg["d_cnt"][:], ccp[:])
                    th = hp.tile([128, 50], F32, tag="th")
                    _ts(nc.vector, th[:], ccp[:], -LAM2, ALU.mult)
                    hh = hp.tile([128, 50], BF16, tag="hh")
                    nc.vector.tensor_copy(hh[:], th[:])
                    ll = hp.tile([128, 50], BF16, tag="ll")
                    nc.vector.tensor_sub(ll[:], th[:], hh[:])
                    nc.sync.dma_start(F[49:50, 800:800 + Q], hh[:])
                    nc.sync.dma_start(F[50:51, 800:800 + Q], ll[:])

                # ---- phase 3: s' matmuls, row min, exp-sum, loss ----
                SPW = 1024
                nsub = (Q + SPW - 1) // SPW
                wdump = mp_.tile([128, SPW], BF16, tag="wdump")
                mparts = mp_.tile([128, nsub], F32, tag="mparts")
                sparts = mp_.tile([128, nsub], F32, tag="sparts")
                with tc.tile_pool(name="ph3a", bufs=2, space="PSUM") as pa, \
                     tc.tile_pool(name="ph3b", bufs=2, space="PSUM") as pb:
                    for ci, (c0, pc) in enumerate(CHUNKS):
                        for si in range(nsub):
                            o = si * SPW
                            w = min(SPW, Q - o)
                            ps = pa.tile([128, SPW], F32, tag="s3")
                            for j in range(0, w, 512):
                                wj = min(512, w - j)
                                nc.tensor.matmul(
                                    ps[0:pc, j:j + wj], F[0:KA, c0:c0 + pc],
                                    F[0:KA, 800 + o + j:800 + o + j + wj],
                                    start=True, stop=True)
                            nc.vector.tensor_reduce(
                                mparts[0:pc, si:si + 1], ps[0:pc, 0:w],
                                axis=AXX, op=ALU.max)
                        mxp = small[:, 24 + ci:25 + ci]
                        nc.vector.tensor_reduce(mxp[0:pc, :], mparts[0:pc, :],
                                                axis=AXX, op=ALU.max)
                        m_ = small[:, 32 + ci:33 + ci]
                        _ts(nc.vector, m_[0:pc, :], mxp[0:pc, :], -0.5, ALU.mult,
                            0.5, ALU.add)
                        a_ = small[:, 40 + ci:41 + ci]
                        nc.vector.reciprocal(a_[0:pc, :], m_[0:pc, :])
                        b_ = small[:, 48 + ci:49 + ci]
                        _ts(nc.vector, b_[0:pc, :], a_[0:pc, :], -1.0, ALU.mult,
                            2.0, ALU.add)
                        logm = small[:, 56 + ci:57 + ci]
                        nc.vector.scalar_tensor_tensor(
                            logm[0:pc, :], mxp[0:pc, :], a_[0:pc, :], b_[0:pc, :],
                            ALU.mult, ALU.add)
                        for si in range(nsub):
                            o = si * SPW
                            w = min(SPW, Q - o)
                            ps = pb.tile([128, SPW], F32, tag="s4")
                            for j in range(0, w, 512):
                                wj = min(512, w - j)
                                nc.tensor.matmul(
                                    ps[0:pc, j:j + wj], F[0:KA, c0:c0 + pc],
                                    F[0:KA, 800 + o + j:800 + o + j + wj],
                                    start=True, stop=True)
                            nc.scalar.activation(
                                wdump[0:pc, 0:w], ps[0:pc, 0:w], ACTF.Exp,
                                bias=b_[0:pc, :], scale=a_[0:pc, :],
                                accum_out=sparts[0:pc, si:si + 1])
                        S_ = small[:, 64 + ci:65 + ci]
                        nc.vector.tensor_reduce(S_[0:pc, :], sparts[0:pc, :],
                                                axis=AXX, op=ALU.add)
                        lnS = small[:, 72 + ci:73 + ci]
                        nc.scalar.activation(lnS[0:pc, :], S_[0:pc, :], ACTF.Ln)
                        lc = small[:, 80 + ci:81 + ci]
                        nc.vector.tensor_sub(lc[0:pc, :], lnS[0:pc, :],
                                             logm[0:pc, :])
                        nc.vector.tensor_add(loss_acc[0:pc, :],
                                             loss_acc[0:pc, :], lc[0:pc, :])

                # ---- final reduce + all-reduce ----
                with tc.tile_pool(name="fin", bufs=1) as fp, \
                     tc.tile_pool(name="finps", bufs=1, space="PSUM") as fps:
                    tot = fps.tile([1, 1], F32, tag="tot")
                    nc.tensor.matmul(tot[:], loss_acc[:], ones_c[:],
                                     start=True, stop=True)
                    tsb = fp.tile([1, 1], F32, tag="tsb")
                    nc.vector.tensor_copy(tsb[:], tot[:])
                    nc.sync.dma_start(ls_in[:].unsqueeze(0), tsb[:])
                    nc.gpsimd.collective_compute(
                        "AllReduce", ALU.add, replica_groups=groups,
                        ins=[ls_in[:].opt()], outs=[ls_out[:].opt()])
                    res = fp.tile([1, 1], F32, tag="res")
                    nc.sync.dma_start(res[:], ls_out[:].unsqueeze(0))
                    if dbg:
                        nc.sync.dma_start(dbg["d_sc"][:], small[:])
                    _ts(nc.vector, res[:], res[:], 1.0 / Q, ALU.mult)
                    nc.sync.dma_start(out_t[:], res[:])

    nc.compile()
    return nc


_NC = None


def _get_nc():
    global _NC
    if _NC is None:
        _NC = build_program()
    return _NC


def make_in_maps(target_features, refer_features, target_field, refer_field):
    timg_np = np.ascontiguousarray(
        np.asarray(target_features, np.float32).reshape(512, 512))
    rimg_np = np.ascontiguousarray(
        np.asarray(refer_features, np.float32).reshape(512, 512))
    tf = np.ascontiguousarray(np.asarray(target_field, np.float32).reshape(-1, 2))
    rf = np.ascontiguousarray(np.asarray(refer_field, np.float32).reshape(-1, 2))
    in_maps = []
    for k in range(NCORES):
        in_maps.append({
            "timg": timg_np,
            "rimg": rimg_np,
            "tfield": np.ascontiguousarray(tf[k * ROWS:(k + 1) * ROWS]),
            "rfield": np.ascontiguousarray(rf[k * ROWS:(k + 1) * ROWS]),
        })
    return in_maps


LAST_RESULTS = None


def kernel(target_features, refer_features, target_field, refer_field,
           args=None, **_ignored):
    global LAST_RESULTS
    from concourse import bass_utils
    nc = _get_nc()
    in_maps = make_in_maps(target_features, refer_features,
                           target_field, refer_field)
    res = bass_utils.run_bass_kernel_spmd(
        nc, in_maps, core_ids=list(range(NCORES)),
        trace=bool(int(os.environ.get("AGC_TRACE", "0"))))
    LAST_RESULTS = res
    return np.asarray(res.results[0]["out"], np.float32).reshape(())


if __name__ == "__main__":
    if "--build" in sys.argv:
        build_program()
        print("BUILD OK")


# revision 33
# speedup vs baseline: 1.4716x; 1.2053x over previous
"""AGC loss kernel for 8 Trainium2 NeuronCores (Bass/Tile).

Self-contained: builds the Bass program, shards inputs host-side, runs via
run_bass_kernel_spmd, returns the full (scalar) output.

Device mapping: target rows p sharded 800/core; sim = bf16 matmul with f32
PSUM; occurrence term folded into the matmul via two extra bf16 contraction
rows (hi/lo split); exact first-index argmax via max/max_index on f32 sim.
"""
import os
import sys
import numpy as np

for _p in ("/opt/trn_rl_repo", os.environ.get("TRN_RL_REPO", "")):
    if _p and _p not in sys.path and os.path.isdir(_p):
        sys.path.insert(0, _p)

import concourse.bass as bass
import concourse.mybir as mybir
from concourse import bacc, tile
from concourse import tile_utils

F32 = mybir.dt.float32
BF16 = mybir.dt.bfloat16
I16 = mybir.dt.int16
I32 = mybir.dt.int32
U32 = mybir.dt.uint32
ALU = mybir.AluOpType
ACTF = mybir.ActivationFunctionType
AXX = mybir.AxisListType.X

HO = 169                 # patch grid 169x169
GRP = 43 * 684           # residue-group stride in the grouped image (29412)
NPOS = 171 * 168 + 168 + 1   # 28897 table elems per channel (g = 171*iy + ix)
C = 49                   # channels (7x7 patch)
KA = 51                  # + 2 aug rows (occ hi/lo)
Q = 6400                 # refer samples (columns)
NCORES = 8
ROWS = Q // NCORES       # 800 target rows per core
CHUNKS = [(i * 128, min(128, ROWS - i * 128)) for i in range((ROWS + 127) // 128)]
NCH = len(CHUNKS)
LAM2 = 0.1               # 2 * LAMBDA_OCC
RNE = float(2 ** 23)

# channel order grouped by e = dx % 3 (must match nothing host-side; any
# consistent permutation works since all downstream ops reduce over channels)
E_GROUPS = [(e, [dx for dx in range(7) if dx % 3 == e]) for e in range(3)]
CNT_E = [171, 171, 170]  # compacted column count per e


def _ts(eng, out, in0, s1, op0, s2=None, op1=None):
    return eng.tensor_scalar(out, in0, s1, s2,
                             op0, op1 if op1 is not None else ALU.bypass)


def build_program(debug_taps=False):
    tile_utils.max_sbuf_usage = 204 * 1024
    nc = bacc.Bacc("TRN2", target_bir_lowering=False, debug=False,
                   num_devices=NCORES)
    dbg = {}
    if debug_taps:
        dbg = {
            "d_gidx": nc.dram_tensor("d_gidx", [64, 400], F32, kind="ExternalOutput"),
            "d_feat": nc.dram_tensor("d_feat", [64, 7200], F32, kind="ExternalOutput"),
            "d_F": nc.dram_tensor("d_F", [64, 7200], F32, kind="ExternalOutput"),
            "d_s0": nc.dram_tensor("d_s0", [128, Q], F32, kind="ExternalOutput"),
            "d_idx": nc.dram_tensor("d_idx", [128, 16], F32, kind="ExternalOutput"),
            "d_cnt": nc.dram_tensor("d_cnt", [128, 50], F32, kind="ExternalOutput"),
            "d_sc": nc.dram_tensor("d_sc", [128, 96], F32, kind="ExternalOutput"),
            "d_ym": nc.dram_tensor("d_ym", [64, 1], F32, kind="ExternalOutput"),
            "d_n2": nc.dram_tensor("d_n2", [1, 7296], F32, kind="ExternalOutput"),
            "d_rs": nc.dram_tensor("d_rs", [128, 57], F32, kind="ExternalOutput"),
        }

    timg = nc.dram_tensor("timg", [512, 512], F32, kind="ExternalInput")
    rimg = nc.dram_tensor("rimg", [512, 512], F32, kind="ExternalInput")
    tfield = nc.dram_tensor("tfield", [ROWS, 2], F32, kind="ExternalInput")
    rfield = nc.dram_tensor("rfield", [ROWS, 2], F32, kind="ExternalInput")
    rg_in = nc.dram_tensor("rg_in", [C * ROWS], F32)
    rg_out = nc.dram_tensor("rg_out", [NCORES * C * ROWS], F32,
                            addr_space="Shared")
    out_t = nc.dram_tensor("out", [1, 1], F32, kind="ExternalOutput")

    ghbm = {n: nc.dram_tensor(f"ghbm_{n}", [3 * GRP], BF16) for n in "tr"}
    sh_dram = {n: nc.dram_tensor(f"shd_{n}", [ROWS], F32) for n in "tr"}
    img_pad = {n: nc.dram_tensor(f"imgpad_{n}", [520 * 512], F32) for n in "tr"}
    cc_in = nc.dram_tensor("cc_in", [Q], F32)
    cc_out = nc.dram_tensor("cc_out", [Q], F32, addr_space="Shared")
    ls_in = nc.dram_tensor("ls_in", [1], F32)
    ls_out = nc.dram_tensor("ls_out", [1], F32, addr_space="Shared")
    groups = [list(range(NCORES))]

    with tile.TileContext(nc) as tc:
        with tc.tile_pool(name="persist", bufs=1) as pp:
            F = pp.tile([64, 800 + Q], BF16, tag="F")        # xf | yf (+aug rows)
            iotaA = pp.tile([128, 50], F32, tag="iA")
            iotaB = pp.tile([128, 128], F32, tag="iB")
            ones_r = pp.tile([1, 64], F32, tag="ones_r")     # K=1 bcast lhsT (f32)
            ones_n2 = pp.tile([64, 1], BF16, tag="ones_n2")  # n2 lhsT (bf16)
            ones_c = pp.tile([128, 1], F32, tag="ones_c")
            loss_acc = pp.tile([128, 1], F32, tag="lacc")

            i32tmp = pp.tile([128, 128], I32, tag="i32tmp")
            nc.gpsimd.iota(i32tmp[:, 0:50], pattern=[[1, 50]], channel_multiplier=0)
            nc.vector.tensor_copy(iotaA[:], i32tmp[:, 0:50])
            nc.gpsimd.iota(i32tmp[:], pattern=[[1, 128]], channel_multiplier=0)
            nc.vector.tensor_copy(iotaB[:], i32tmp[:])
            nc.vector.memset(ones_r[:], 1.0)
            nc.vector.memset(ones_n2[:], 1.0)
            nc.vector.memset(ones_c[:], 1.0)
            nc.vector.memset(loss_acc[:], 0.0)
            # aug rows: zero rows 32-63 (rows 32-48 later overwritten by xf/yf
            # write), then ones into xf-aug via DMA (DVE can't start at p=49)
            nc.vector.memset(F[32:64, :], 0.0)
            aug2 = pp.tile([2, 800], BF16, tag="aug2")
            nc.vector.memset(aug2[:], 1.0)
            nc.sync.dma_start(F[49:51, 0:800], aug2[:])

            # =========== prologue: gather + normalize ====================
            with tc.tile_pool(name="pro", bufs=1) as pro:
                featcat = pro.tile([64, 800 + Q], F32, tag="featcat")

                with tc.tile_pool(name="progA", bufs=1) as gp:
                    NPOSW = (NPOS + 3) // 2  # u32 words (pad to even+1)
                    apt = gp.tile([64, 2 * NPOSW], BF16, tag="apt")
                    nc.gpsimd.memset(apt[32:64, :], 0.0)

                    # ---- grid-sample indices (target shard + refer shard) ----
                    gidx = {}
                    for name, fld in (("t", tfield), ("r", rfield)):
                        nw = ROWS // 16
                        wx = gp.tile([16, ROWS // 16], F32, tag="wx")
                        wy = gp.tile([16, ROWS // 16], F32, tag="wy")
                        fv = fld[:].rearrange("(j p) c -> p j c", p=16)
                        nc.sync.dma_start(wx[:], fv[:, :, 0:1].squeeze(2))
                        nc.sync.dma_start(wy[:], fv[:, :, 1:2].squeeze(2))
                        for w in (wx, wy):
                            u = w[:, 0:nw]
                            _ts(nc.vector, u, u, 2.0, ALU.mult, 1.0, ALU.subtract)
                            _ts(nc.vector, u, u, 1.0, ALU.add, float(HO), ALU.mult)
                            _ts(nc.vector, u, u, 1.0, ALU.subtract, 0.5, ALU.mult)
                            _ts(nc.vector, u, u, RNE, ALU.add)
                            _ts(nc.vector, u, u, RNE, ALU.subtract)
                            _ts(nc.vector, u, u, 0.0, ALU.max, float(HO - 1), ALU.min)
                        nc.vector.scalar_tensor_tensor(
                            wy[:, 0:nw], wy[:, 0:nw], 171.0, wx[:, 0:nw],
                            ALU.mult, ALU.add)
                        if dbg and name == "r":
                            nc.sync.dma_start(dbg["d_gidx"][0:16, 0:nw], wy[:, 0:nw])
                        # word index j = floor(g/2)
                        wj = gp.tile([16, ROWS // 16], F32, tag="wjf")
                        _ts(nc.vector, wj[:], wy[:, 0:nw], 0.5, ALU.mult, 0.25,
                            ALU.subtract)
                        _ts(nc.vector, wj[:], wj[:], RNE, ALU.add)
                        _ts(nc.vector, wj[:], wj[:], RNE, ALU.subtract)
                        gi = gp.tile([64, ROWS // 16], I16, tag=f"gi_{name}")
                        nc.vector.tensor_copy(gi[0:16, 0:nw], wj[:])
                        for k in range(1, 4):
                            nc.sync.dma_start(gi[16 * k:16 * k + 16, 0:nw],
                                              gi[0:16, 0:nw])
                        # shamt = 16*(1-parity) in partition-major [50,16]
                        # layout (s = 16p + j) for a contiguous DRAM riffle
                        px = gp.tile([50, 16], F32, tag="px")
                        py = gp.tile([50, 16], F32, tag="py")
                        fv2 = fld[:].rearrange("(p j) c -> p j c", p=50)
                        nc.sync.dma_start(px[:], fv2[:, :, 0:1].squeeze(2))
                        nc.sync.dma_start(py[:], fv2[:, :, 1:2].squeeze(2))
                        for w in (px, py):
                            u = w[:]
                            _ts(nc.vector, u, u, 2.0, ALU.mult, 1.0, ALU.subtract)
                            _ts(nc.vector, u, u, 1.0, ALU.add, float(HO), ALU.mult)
                            _ts(nc.vector, u, u, 1.0, ALU.subtract, 0.5, ALU.mult)
                            _ts(nc.vector, u, u, RNE, ALU.add)
                            _ts(nc.vector, u, u, RNE, ALU.subtract)
                            _ts(nc.vector, u, u, 0.0, ALU.max, float(HO - 1), ALU.min)
                        nc.vector.scalar_tensor_tensor(
                            py[:], py[:], 171.0, px[:], ALU.mult, ALU.add)
                        pj = gp.tile([50, 16], F32, tag="pj")
                        _ts(nc.vector, pj[:], py[:], 0.5, ALU.mult, 0.25,
                            ALU.subtract)
                        _ts(nc.vector, pj[:], pj[:], RNE, ALU.add)
                        _ts(nc.vector, pj[:], pj[:], RNE, ALU.subtract)
                        # shamt = 16 + 16*(2j - g)
                        _ts(nc.vector, pj[:], pj[:], 2.0, ALU.mult)
                        nc.vector.tensor_sub(pj[:], pj[:], py[:])
                        _ts(nc.vector, pj[:], pj[:], 16.0, ALU.mult, 16.0, ALU.add)
                        nc.sync.dma_start(
                            sh_dram[name][:].rearrange("(p j) -> p j", p=50),
                            pj[:])
                        shrow = gp.tile([1, ROWS], F32, tag=f"shrow_{name}")
                        nc.sync.dma_start(shrow[:],
                                          sh_dram[name][:].unsqueeze(0))
                        gidx[name] = (gi, shrow)

                    # channel order: (e, rho, dyq, dxq)
                    rsh = gp.tile([64, ROWS], F32, tag="rsh")
                    qi = 0
                    for iname in ("r", "t"):
                        img = rimg if iname == "r" else timg
                        gh = ghbm[iname]
                        ipad = img_pad[iname]
                        # pad image to 520 rows in HBM
                        nc.sync.dma_start(
                            ipad[0:512 * 512].unsqueeze(0),
                            img[:].rearrange("a b -> (a b)").unsqueeze(0))
                        nc.scalar.dma_start(
                            ipad[512 * 512:520 * 512].unsqueeze(0),
                            img[:].rearrange("a b -> (a b)")[0:8 * 512].unsqueeze(0))
                        # residue-grouped staging: partitions (m,q) hold rows
                        # 3*(4q+k)+m (k=0..3), m in {0,1} in stg01, m=2 in stg2
                        stg01 = gp.tile([86, 4, 512], F32, tag="stg01")
                        stg2 = gp.tile([43, 4, 512], F32, tag="stg2")
                        for m in range(3):
                            srcap = bass.AP(tensor=ipad, offset=m * 512,
                                            ap=[[12 * 512, 43], [3 * 512, 4],
                                                [1, 512]])
                            dstt = stg01[43 * m:43 * m + 43] if m < 2 else stg2[:]
                            eng = nc.sync if qi % 2 == 0 else nc.scalar
                            eng.dma_start(dstt, srcap)
                            qi += 1
                        # per e: column-compact (DVE) then grouped write to HBM
                        # gh layout: [e-interleaved? no]: gh holds for each e a
                        # region of 3*GRP/..; we use separate offsets per e in
                        # THREE dram tensors? simpler: gh covers ONE e at a time
                        # is wrong -- instead allocate gh as [3e][3m][GRP] via
                        # offset e*3*GRP... but gh was sized [3*GRP]. We only
                        # need e's data until its apt DMA is done; process e
                        # sequentially reusing gh regions [m*GRP].
                        pbase = 0
                        for e, dxs in E_GROUPS:
                            cnt = CNT_E[e]
                            ce01 = gp.tile([86, 4, 171], BF16, tag="ce01")
                            ce2 = gp.tile([43, 4, 171], BF16, tag="ce2")
                            if cnt < 171:
                                nc.vector.memset(ce01[:, :, cnt:171], 0.0)
                                nc.vector.memset(ce2[:, :, cnt:171], 0.0)
                            nc.vector.tensor_copy(
                                ce01[:, :, 0:cnt],
                                stg01[:, :, e:e + 3 * (cnt - 1) + 1:3])
                            nc.vector.tensor_copy(
                                ce2[:, :, 0:cnt],
                                stg2[:, :, e:e + 3 * (cnt - 1) + 1:3])
                            for m in range(3):
                                srct = (ce01[43 * m:43 * m + 43] if m < 2
                                        else ce2[:])
                                dstap = bass.AP(tensor=gh, offset=m * GRP,
                                                ap=[[684, 43], [1, 684]])
                                eng = nc.sync if qi % 2 == 0 else nc.scalar
                                eng.dma_start(dstap, srct)
                                qi += 1
                            # table slices: channels (rho, dyq, dxq) for this e
                            for rho in range(3):
                                dys = [dy for dy in range(7) if dy % 3 == rho]
                                nch = len(dys) * len(dxs)
                                srcap = bass.AP(
                                    tensor=gh,
                                    offset=rho * GRP,
                                    ap=[[171, len(dys)], [1, len(dxs)],
                                        [1, 2 * NPOSW]])
                                eng = nc.sync if qi % 2 == 0 else nc.scalar
                                eng.dma_start(
                                    apt[pbase:pbase + nch, 0:2 * NPOSW], srcap)
                                qi += 1
                                pbase += nch
                        assert pbase == C
                        # gather this image's shard of samples (u32 pairs)
                        gi, shrow = gidx[iname]
                        outt = (rsh[:, :] if iname == "r"
                                else featcat[:, 0:ROWS])
                        gout = gp.tile([64, ROWS], I32, tag="gout")
                        nc.gpsimd.ap_gather(
                            out_ap=gout[:].unsqueeze(2),
                            in_ap=apt[:].bitcast(I32).unsqueeze(2),
                            idxs_ap=gi[:, 0:ROWS // 16],
                            channels=64, num_elems=NPOSW, d=1, num_idxs=ROWS)
                        # replicate shamt row to 64 partitions via K=1 matmul
                        with tc.tile_pool(name="shps", bufs=1,
                                          space="PSUM") as shp:
                            shps = shp.tile([64, ROWS], F32, tag="shps")
                            for j0 in range(0, ROWS, 512):
                                wj0 = min(512, ROWS - j0)
                                nc.tensor.matmul(shps[:, j0:j0 + wj0],
                                                 ones_r[0:1, :],
                                                 shrow[:, j0:j0 + wj0],
                                                 start=True, stop=True)
                            shi = gp.tile([64, ROWS], I32, tag="shi")
                            nc.vector.tensor_copy(shi[:], shps[:])
                        nc.vector.tensor_tensor(
                            gout[:], gout[:], shi[:],
                            op=ALU.logical_shift_left)
                        _ts(nc.vector, gout[:], gout[:], -65536,
                            ALU.bitwise_and)
                        nc.vector.tensor_copy(outt, gout[:].bitcast(F32))

                    # ---- all-gather refer features across cores ----
                    nc.sync.dma_start(
                        rg_in[:].rearrange("(c s) -> c s", c=C), rsh[0:C, :])
                    nc.gpsimd.collective_compute(
                        "AllGather", ALU.bypass, replica_groups=groups,
                        ins=[rg_in[:].opt()], outs=[rg_out[:].opt()])
                    srcap = bass.AP(tensor=rg_out, offset=0,
                                    ap=[[ROWS, C], [C * ROWS, NCORES],
                                        [1, ROWS]])
                    dstap = featcat[0:C, 800:800 + Q].rearrange(
                        "c (k s) -> c k s", k=NCORES)
                    nc.sync.dma_start(dstap, srcap)

                # ---- y_mean (refer), centered features ----
                if dbg:
                    nc.sync.dma_start(dbg["d_feat"][:], featcat[:])
                _gpcm = tc.tile_pool(name="progB", bufs=1)
                gp = _gpcm.__enter__()
                ymean = gp.tile([64, 1], F32, tag="ymean")
                nc.vector.memset(ymean[:], 0.0)
                nc.scalar.activation(featcat[0:C, 800:800 + Q],
                                     featcat[0:C, 800:800 + Q],
                                     ACTF.Copy, accum_out=ymean[0:C, :])
                if dbg:
                    nc.sync.dma_start(dbg["d_ym"][:], ymean[:])
                _ts(nc.vector, ymean[0:C, :], ymean[0:C, :], 1.0 / Q, ALU.mult)
                _ts(nc.vector, featcat[0:C, :], featcat[0:C, :],
                    ymean[0:C, :], ALU.subtract)

                # ---- column norms ----
                NT = 800 + Q
                sq = gp.tile([64, NT], BF16, tag="sq")
                nc.scalar.activation(sq[0:C, :], featcat[0:C, :], ACTF.Square)
                n2row = gp.tile([1, 7296], F32, tag="n2row")
                nc.vector.memset(n2row[:], 1.0)
                nsl = [(i * 512, min(512, NT - i * 512))
                       for i in range((NT + 511) // 512)]
                with tc.tile_pool(name="n2ps", bufs=2, space="PSUM") as n2p:
                    for r0 in range(0, len(nsl), 4):
                        sls = nsl[r0:r0 + 4]
                        ps = n2p.tile([1, 2048], F32, tag="n2psum")
                        for j, (o, w) in enumerate(sls):
                            nc.tensor.matmul(ps[:, j * 512:j * 512 + w],
                                             ones_n2[0:C, :], sq[0:C, o:o + w],
                                             start=True, stop=True)
                        o0 = sls[0][0]
                        wtot = sum(w for _, w in sls)
                        nc.scalar.activation(n2row[:, o0:o0 + wtot],
                                             ps[:, 0:wtot], ACTF.Copy)
                if dbg:
                    nc.sync.dma_start(dbg["d_n2"][:], n2row[:])
                # compact rsqrt with one Newton step
                cpt = gp.tile([128, 57], F32, tag="cpt")
                nc.sync.dma_start(
                    cpt[:], n2row[:].rearrange("a (p j) -> a p j", p=128))
                rc = gp.tile([128, 57], F32, tag="rc")
                nc.vector.reciprocal(rc[:], cpt[:])
                rs = gp.tile([128, 57], F32, tag="rs")
                nc.scalar.activation(rs[:], rc[:], ACTF.Sqrt)
                t2 = gp.tile([128, 57], F32, tag="t2")
                nc.vector.tensor_mul(t2[:], rs[:], rs[:])
                nc.vector.tensor_mul(t2[:], t2[:], cpt[:])
                _ts(nc.vector, t2[:], t2[:], -0.5, ALU.mult, 1.5, ALU.add)
                nc.vector.tensor_mul(rs[:], rs[:], t2[:])
                if dbg:
                    nc.sync.dma_start(dbg["d_rs"][:], rs[:])
                nc.sync.dma_start(
                    n2row[:].rearrange("a (p j) -> a p j", p=128), rs[:])
                # broadcast 1/norm to C partitions; write xf/yf bf16
                with tc.tile_pool(name="bcps", bufs=2, space="PSUM") as bp:
                    for o in range(0, NT, 2048):
                        w = min(2048, NT - o)
                        ps = bp.tile([64, 2048], F32, tag="bc")
                        for j in range(0, w, 512):
                            wj = min(512, w - j)
                            nc.tensor.matmul(ps[0:C, j:j + wj], ones_r[0:1, 0:C],
                                             n2row[:, o + j:o + j + wj],
                                             start=True, stop=True)
                        nc.vector.tensor_mul(F[0:C, o:o + w],
                                             featcat[0:C, o:o + w], ps[0:C, 0:w])
                if dbg:
                    nc.gpsimd.dma_start(dbg["d_F"][:], F[:])
                _gpcm.__exit__(None, None, None)

            # =========== main phases =====================================
            with tc.tile_pool(name="mid", bufs=1) as mp_:
                ohA = mp_.tile([128, NCH * 50], BF16, tag="ohA")
                ohB = mp_.tile([128, NCH * 128], BF16, tag="ohB")
                val8 = mp_.tile([128, 8 * NCH], F32, tag="val8")
                idx8 = mp_.tile([128, 8 * NCH], U32, tag="idx8")
                small = mp_.tile([128, 96], F32, tag="small")
                nc.vector.memset(small[:], 0.0)

                # ---- phase 1: sim matmul, f32 store, exact argmax ----
                with tc.tile_pool(name="ph1", bufs=2) as s1pool, \
                     tc.tile_pool(name="ph1ps", bufs=2, space="PSUM") as p1p:
                    for ci, (c0, pc) in enumerate(CHUNKS):
                        s_sl = s1pool.tile([128, Q], F32, tag="schunk")
                        for o in range(0, Q, 2048):
                            w = min(2048, Q - o)
                            ps = p1p.tile([128, 2048], F32, tag="s1")
                            for j in range(0, w, 512):
                                wj = min(512, w - j)
                                nc.tensor.matmul(
                                    ps[0:pc, j:j + wj], F[0:C, c0:c0 + pc],
                                    F[0:C, 800 + o + j:800 + o + j + wj],
                                    start=True, stop=True)
                            nc.scalar.activation(s_sl[0:pc, o:o + w],
                                                 ps[0:pc, 0:w], ACTF.Copy)
                        if dbg and ci == 0:
                            nc.sync.dma_start(dbg["d_s0"][:], s_sl[:])
                        v8 = val8[:, 8 * ci:8 * ci + 8]
                        i8 = idx8[:, 8 * ci:8 * ci + 8]
                        nc.vector.max(v8[0:pc, :], s_sl[0:pc, :])
                        nc.vector.max_index(i8[0:pc, :], v8[0:pc, :],
                                            s_sl[0:pc, :])
                        qf = small[:, ci:ci + 1]
                        nc.vector.tensor_copy(qf[0:pc, :], i8[0:pc, 0:1])
                        af = small[:, 8 + ci:9 + ci]
                        _ts(nc.vector, af[0:pc, :], qf[0:pc, :], 1.0 / 128.0,
                            ALU.mult, 63.5 / 128.0, ALU.subtract)
                        _ts(nc.vector, af[0:pc, :], af[0:pc, :], RNE, ALU.add)
                        _ts(nc.vector, af[0:pc, :], af[0:pc, :], RNE, ALU.subtract)
                        bf_ = small[:, 16 + ci:17 + ci]
                        nc.vector.scalar_tensor_tensor(
                            bf_[0:pc, :], af[0:pc, :], -128.0, qf[0:pc, :],
                            ALU.mult, ALU.add)
                        _ts(nc.vector, ohA[0:pc, ci * 50:(ci + 1) * 50],
                            iotaA[0:pc, :], af[0:pc, :], ALU.is_equal)
                        _ts(nc.vector, ohB[0:pc, ci * 128:(ci + 1) * 128],
                            iotaB[0:pc, :], bf_[0:pc, :], ALU.is_equal)

                # ---- phase 2: histogram, all-reduce, occ rows ----
                with tc.tile_pool(name="ph2", bufs=1) as hp, \
                     tc.tile_pool(name="ph2ps", bufs=1, space="PSUM") as cp:
                    cpsum = cp.tile([64, 128], F32, tag="cpsum")
                    for ci, (c0, pc) in enumerate(CHUNKS):
                        nc.tensor.matmul(cpsum[0:50, :],
                                         ohA[0:pc, ci * 50:(ci + 1) * 50],
                                         ohB[0:pc, ci * 128:(ci + 1) * 128],
                                         start=(ci == 0), stop=(ci == NCH - 1))
                    csb = hp.tile([64, 128], F32, tag="csb")
                    nc.vector.tensor_copy(csb[0:50, :], cpsum[0:50, :])
                    nc.sync.dma_start(
                        cc_in[:].rearrange("(p j) -> p j", p=50), csb[0:50, :])
                    nc.gpsimd.collective_compute(
                        "AllReduce", ALU.add, replica_groups=groups,
                        ins=[cc_in[:].opt()], outs=[cc_out[:].opt()])
                    ccp = hp.tile([128, 50], F32, tag="ccp")
                    nc.sync.dma_start(ccp[:],
                                      cc_out[:].rearrange("(p j) -> p j", p=128))
                    if dbg:
                        nc.sync.dma_start(dbg["d_cnt"][:], ccp[:])
                    th = hp.tile([128, 50], F32, tag="th")
                    _ts(nc.vector, th[:], ccp[:], -LAM2, ALU.mult)
                    hh = hp.tile([128, 50], BF16, tag="hh")
                    nc.vector.tensor_copy(hh[:], th[:])
                    ll = hp.tile([128, 50], BF16, tag="ll")
                    nc.vector.tensor_sub(ll[:], th[:], hh[:])
                    nc.sync.dma_start(F[49:50, 800:800 + Q], hh[:])
                    nc.sync.dma_start(F[50:51, 800:800 + Q], ll[:])

                # ---- phase 3: s' matmuls, row min, exp-sum, loss ----
                SPW = 1024
                nsub = (Q + SPW - 1) // SPW
                mparts = mp_.tile([128, nsub * NCH], F32, tag="mparts")
                sparts = mp_.tile([128, nsub * NCH], F32, tag="sparts")
                with tc.tile_pool(name="ph3a", bufs=2, space="PSUM") as pa, \
                     tc.tile_pool(name="ph3b", bufs=2, space="PSUM") as pb, \
                     tc.tile_pool(name="ph3w", bufs=2) as pw:
                    for ci, (c0, pc) in enumerate(CHUNKS):
                        for si in range(nsub):
                            o = si * SPW
                            w = min(SPW, Q - o)
                            ps = pa.tile([128, SPW], F32, tag="s3")
                            for j in range(0, w, 512):
                                wj = min(512, w - j)
                                nc.tensor.matmul(
                                    ps[0:pc, j:j + wj], F[0:KA, c0:c0 + pc],
                                    F[0:KA, 800 + o + j:800 + o + j + wj],
                                    start=True, stop=True)
                            nc.vector.tensor_reduce(
                                mparts[0:pc, nsub * ci + si:nsub * ci + si + 1],
                                ps[0:pc, 0:w], axis=AXX, op=ALU.max)
                        mxp = small[:, 24 + ci:25 + ci]
                        nc.vector.tensor_reduce(
                            mxp[0:pc, :],
                            mparts[0:pc, nsub * ci:nsub * (ci + 1)],
                            axis=AXX, op=ALU.max)
                        m_ = small[:, 32 + ci:33 + ci]
                        _ts(nc.vector, m_[0:pc, :], mxp[0:pc, :], -0.5, ALU.mult,
                            0.5, ALU.add)
                        a_ = small[:, 40 + ci:41 + ci]
                        nc.vector.reciprocal(a_[0:pc, :], m_[0:pc, :])
                        b_ = small[:, 48 + ci:49 + ci]
                        _ts(nc.vector, b_[0:pc, :], a_[0:pc, :], -1.0, ALU.mult,
                            2.0, ALU.add)
                        logm = small[:, 56 + ci:57 + ci]
                        nc.vector.scalar_tensor_tensor(
                            logm[0:pc, :], mxp[0:pc, :], a_[0:pc, :], b_[0:pc, :],
                            ALU.mult, ALU.add)
                        for si in range(nsub):
                            o = si * SPW
                            w = min(SPW, Q - o)
                            ps = pb.tile([128, SPW], F32, tag="s4")
                            for j in range(0, w, 512):
                                wj = min(512, w - j)
                                nc.tensor.matmul(
                                    ps[0:pc, j:j + wj], F[0:KA, c0:c0 + pc],
                                    F[0:KA, 800 + o + j:800 + o + j + wj],
                                    start=True, stop=True)
                            wdump = pw.tile([128, SPW], BF16, tag="wdump")
                            nc.scalar.activation(
                                wdump[0:pc, 0:w], ps[0:pc, 0:w], ACTF.Exp,
                                bias=b_[0:pc, :], scale=a_[0:pc, :],
                                accum_out=sparts[0:pc,
                                                 nsub * ci + si:nsub * ci + si + 1])
                        S_ = small[:, 64 + ci:65 + ci]
                        nc.vector.tensor_reduce(
                            S_[0:pc, :], sparts[0:pc, nsub * ci:nsub * (ci + 1)],
                            axis=AXX, op=ALU.add)
                        lnS = small[:, 72 + ci:73 + ci]
                        nc.scalar.activation(lnS[0:pc, :], S_[0:pc, :], ACTF.Ln)
                        lc = small[:, 80 + ci:81 + ci]
                        nc.vector.tensor_sub(lc[0:pc, :], lnS[0:pc, :],
                                             logm[0:pc, :])
                        nc.vector.tensor_add(loss_acc[0:pc, :],
                                             loss_acc[0:pc, :], lc[0:pc, :])

                # ---- final reduce + all-reduce ----
                with tc.tile_pool(name="fin", bufs=1) as fp, \
                     tc.tile_pool(name="finps", bufs=1, space="PSUM") as fps:
                    tot = fps.tile([1, 1], F32, tag="tot")
                    nc.tensor.matmul(tot[:], loss_acc[:], ones_c[:],
                                     start=True, stop=True)
                    tsb = fp.tile([1, 1], F32, tag="tsb")
                    nc.vector.tensor_copy(tsb[:], tot[:])
                    nc.sync.dma_start(ls_in[:].unsqueeze(0), tsb[:])
                    nc.gpsimd.collective_compute(
                        "AllReduce", ALU.add, replica_groups=groups,
                        ins=[ls_in[:].opt()], outs=[ls_out[:].opt()])
                    res = fp.tile([1, 1], F32, tag="res")
                    nc.sync.dma_start(res[:], ls_out[:].unsqueeze(0))
                    if dbg:
                        nc.sync.dma_start(dbg["d_sc"][:], small[:])
                    _ts(nc.vector, res[:], res[:], 1.0 / Q, ALU.mult)
                    nc.sync.dma_start(out_t[:], res[:])

    nc.compile()
    return nc


_NC = None


def _get_nc():
    global _NC
    if _NC is None:
        _NC = build_program()
    return _NC


def make_in_maps(target_features, refer_features, target_field, refer_field):
    timg_np = np.ascontiguousarray(
        np.asarray(target_features, np.float32).reshape(512, 512))
    rimg_np = np.ascontiguousarray(
        np.asarray(refer_features, np.float32).reshape(512, 512))
    tf = np.ascontiguousarray(np.asarray(target_field, np.float32).reshape(-1, 2))
    rf = np.ascontiguousarray(np.asarray(refer_field, np.float32).reshape(-1, 2))
    in_maps = []
    for k in range(NCORES):
        in_maps.append({
            "timg": timg_np,
            "rimg": rimg_np,
            "tfield": np.ascontiguousarray(tf[k * ROWS:(k + 1) * ROWS]),
            "rfield": np.ascontiguousarray(rf[k * ROWS:(k + 1) * ROWS]),
        })
    return in_maps


LAST_RESULTS = None


def kernel(target_features, refer_features, target_field, refer_field,
           args=None, **_ignored):
    global LAST_RESULTS
    from concourse import bass_utils
    nc = _get_nc()
    in_maps = make_in_maps(target_features, refer_features,
                           target_field, refer_field)
    res = bass_utils.run_bass_kernel_spmd(
        nc, in_maps, core_ids=list(range(NCORES)),
        trace=bool(int(os.environ.get("AGC_TRACE", "0"))))
    LAST_RESULTS = res
    return np.asarray(res.results[0]["out"], np.float32).reshape(())


if __name__ == "__main__":
    if "--build" in sys.argv:
        build_program()
        print("BUILD OK")
